# revision 21
# baseline (speedup 1.0000x reference)
"""Gated linear attention kernel for one TRN2 chip (8 NeuronCores).

Math (see reference):
    q = elu(X Wq)+1, k = elu(X Wk)+1, v = X Wv, g = X Wg
    qk = sum_d(q*k) per head; k_sum = sum_seq(k); norm = sum_d(q*k_sum)
    z = qk*v/(norm+1e-6); z = LayerNorm(z)*gamma+beta; out = (z*silu(g)) Wo

Sharding: data-parallel over the 16384 tokens, 2048 per core; cores 0-3 own
batch 0, cores 4-7 batch 1.  The only cross-core coupling is k_sum (a [1,1024]
vector per batch) -> AllReduce within 4-core groups.

The kernel is PE-streaming-bound at the GPIO-throttled 1.95 GHz clock (the
13/16 clock-gate engages ~60us in; MMs pipeline at 263ns/512cols), so v2/v3
cut PE cycles and then keep every other engine strictly under the PE:
  * q/k projections in fp8-e4m3 perf_mode=DoubleRow: 2 fp8 MACs/cell/cycle,
    pairing adjacent 128-row k-slices via 3D APs [128,2,M]/[128,2,N]
    (HW-verified, 8e-4).  Host pre-scales Wq/Wk by 32; the 1/32 descale folds
    into the elu ACTs' scale operand.  End-to-end rel err 6.9e-3 (gate 2e-2);
    the q-quantization error cancels between qk and norm, and k_sum averages
    8192 positive terms.  v/g/o cannot go fp8 (3.5e-2+ each, and u underflows
    e4m3 by ~2^-13).
  * elu = min(exp(x),1)+relu(x) on [128,1024] 2-bank PSUM tiles; exp + half
    the relu on ACT (1.87us/tile), other relu half + combine on DVE
    (1.87us/tile), both under the 2.1us/tile of DR matmuls.
  * qk = sum_d(q*k) deferred to phase 2 where the DVE has slack.
  * The AllReduce takes ~45us trigger-to-done on this stack (mesh latency +
    skew + a one-time ~50us replica-group barrier).  A dummy warm-up
    AllReduce issued at kernel start absorbs the barrier; the real AR chain
    (vector hp copy -> gpsimd dma -> AR -> vector hp f32->bf16 -> gpsimd
    broadcast) fires right after 1a.  Phase 2 gives it ~34us of runway: the
    software pipeline is 4 deep and v PSUM evacuates to a 6-tile rolling
    SBUF buffer via scalar Identity, so v/g matmuls never wait on the
    ksb-gated DVE chain.
  * Phase-2 scalar ops (silu + v-evac) are hard-gated on a zero-bias AP
    derived from 1b's last tile: without it the scheduler interleaves silu
    into phase 1 and thrashes the exp<->silu ACT tables (8x1.3us reloads on
    the bottleneck engine).  Identity/relu live in every table set.
  * y = py*rsig evacuates on ACT (Identity, scale=rsig); u = (z-mu)*s in one
    stt; rsqrt uses 1 Newton step (rel err 1.7e-3 on rsig, ~0.2e-3 on out).
  * SBUF lifetimes: xt8/wq8/wk8 close after 1b; Wo + the v ring live in the
    freed bytes (Wo loads via the gpsimd queue -- a blocked load on the
    scalar queue would deadlock behind the gated silu).
  * Output is stored bf16 (halves write traffic), upcast on the host.
Carried over from v1: X^T/k/q SBUF-resident, DVE rsqrt Newton with 1/sigma
folded past Wo, DMA-XBAR transpose for u^T, k-sliced initial loads, gamma
folded into Wo on the host; beta==0 verified on the host.
"""

import os
from contextlib import ExitStack

import numpy as np

import concourse.bass as bass
import concourse.mybir as mybir
import concourse.tile as tile
from concourse.bass_utils import run_bass_kernel_spmd

F32 = mybir.dt.float32
BF16 = mybir.dt.bfloat16
FP8 = mybir.dt.float8e4
U32 = mybir.dt.uint32
AX = mybir.AxisListType
ALU = mybir.AluOpType
ACT_F = mybir.ActivationFunctionType
DR = mybir.MatmulPerfMode.DoubleRow

H = 1024
NH = 16
DK = 64
N_CORES = 8
WSCALE = 32.0          # host multiplies Wq/Wk by this before e4m3 quantization
ISCALE = 1.0 / WSCALE  # folded into the elu ACTs
PIPE = 4               # phase-2 software pipeline depth (AR runway)
VBUFS = 6              # rolling v ring tiles


def _split_multi_waits(nc, cap=1):
    """walrus in this image rejects instructions with more than ~2 sync waits
    (Tile attaches several to its kernel-tail drain).  Move excess waits onto
    preceding same-engine NoOps."""
    for f in nc.m.functions:
        for bb in f.blocks:
            insts = bb.instructions
            new_list = []
            changed = False
            for inst in insts:
                si = inst.sync_info
                waits = list(si.on_wait) if si else []
                if len(waits) > cap:
                    changed = True
                    for kk, w in enumerate(waits[:-cap]):
                        new_list.append(
                            mybir.InstNoOp(
                                name=f"{inst.name}-wsplit{kk}",
                                engine=inst.engine,
                                ins=[],
                                outs=[],
                                sync_info=mybir.SyncInfo(on_wait=[w], on_update=[]),
                            )
                        )
                    inst.sync_info = mybir.SyncInfo(
                        on_wait=waits[-cap:], on_update=list(si.on_update)
                    )
                new_list.append(inst)
            if changed:
                live = bb.instructions
                live.clear()
                for i in new_list:
                    bb.add_instruction(i)
    return nc


def build_gla(T=2048, groups=((0, 1, 2, 3), (4, 5, 6, 7)), n_devices=8,
              apply_beta=False, split_waits=True, use_silu=True):
    """Build the per-core SPMD program.  T = tokens per core."""
    assert T % 128 == 0
    NT = T // 128      # 128-token tiles
    KT = H // 128      # contraction slices
    KP = KT // 2       # DoubleRow k-pair slices

    nc = bass.Bass(num_devices=n_devices)
    xt_d = nc.declare_dram_parameter("xt", [H, T], BF16, isOutput=False)
    xt8_d = nc.declare_dram_parameter("xt8", [H, T], FP8, isOutput=False)
    wq8_d = nc.declare_dram_parameter("wq8", [H, H], FP8, isOutput=False)
    wk8_d = nc.declare_dram_parameter("wk8", [H, H], FP8, isOutput=False)
    w_d = {
        n: nc.declare_dram_parameter(n, [H, H], BF16, isOutput=False)
        for n in ("wv", "wg", "wo")
    }
    beta_d = (
        nc.declare_dram_parameter("beta", [1, H], BF16, isOutput=False)
        if apply_beta
        else None
    )
    out_d = nc.declare_dram_parameter("out", [T, H], BF16, isOutput=True)

    ks_in = nc.dram_tensor("ks_in", [1, H], F32)
    ks_out = nc.dram_tensor("ks_out", [1, H], F32)
    arw_in = nc.dram_tensor("arw_in", [1, 8], F32)
    arw_out = nc.dram_tensor("arw_out", [1, 8], F32)

    def mm(ps, lhsT, rhs, start, stop):
        nc.tensor.matmul(ps, lhsT=lhsT, rhs=rhs, start=start, stop=stop)

    with tile.TileContext(nc) as tc:
        with (
            tc.tile_pool(name="singles", bufs=1) as singles,
            tc.tile_pool(name="w", bufs=2) as wpool,
            tc.tile_pool(name="xt", bufs=1) as xtpool,
            tc.tile_pool(name="kt", bufs=1) as ktpool,
            tc.tile_pool(name="qt", bufs=1) as qtpool,
            tc.tile_pool(name="small", bufs=3) as smpool,
        ):
            ones_col = singles.tile([128, 1], BF16)
            nc.vector.memset(ones_col, 1.0)
            # rsqrt bit-hack constants (as APs: immediate ints on uint ops
            # are unreliable through the f32 immediate path)
            c_shift1 = singles.tile([128, 1], U32)
            nc.vector.memset(c_shift1, 1)
            c_magic = singles.tile([128, 1], U32)
            nc.vector.memset(c_magic, 0x5F3759DF)

            # warm-up AllReduce: establishes the replica-group barrier +
            # CC stream (~50us, one-time) while phase 1 computes, so the
            # real k_sum AR only pays ring latency.
            arw_sb = singles.tile([1, 8], F32)
            nc.gpsimd.memset(arw_sb, 0.0)
            nc.gpsimd.dma_start(out=arw_in[:, :], in_=arw_sb)
            nc.gpsimd.collective_compute(
                "AllReduce", ALU.add,
                replica_groups=[list(g) for g in groups],
                ins=[arw_in[:, :]], outs=[arw_out[:, :]],
            )

            xt_all = xtpool.tile([128, KT, T], BF16)
            kt_all = ktpool.tile([128, NT, H], BF16)
            qt_all = qtpool.tile([128, NT, H], BF16)
            wv_t = wpool.tile([128, KT, H], BF16, tag="w", name="wv")
            wg_t = wpool.tile([128, KT, H], BF16, tag="w", name="wg")

            def load_w(t, name, engine=None):
                # one wide descriptor: [H, H] viewed as [p, k-slice, cols]
                (engine or nc.sync).dma_start(
                    out=t[:, :, :],
                    in_=w_d[name][:, :].rearrange("(k p) n -> p k n", p=128))

            def elu1(dst, ps):
                # dst = elu(ps/32)+1 = min(exp(ps/32), 1) + relu(ps/32);
                # exp + low relu half on ACT, high relu half + combine on DVE
                e = elupool.tile([128, H], BF16, tag="elue")
                r = elupool.tile([128, H], BF16, tag="elur")
                nc.scalar.activation(out=e, in_=ps, func=ACT_F.Exp,
                                     scale=ISCALE)
                nc.scalar.activation(out=r[:, 0:512], in_=ps[:, 0:512],
                                     func=ACT_F.Relu, scale=ISCALE)
                nc.vector.tensor_scalar(
                    out=r[:, 512:H], in0=ps[:, 512:H],
                    scalar1=ISCALE, scalar2=0.0, op0=ALU.mult, op1=ALU.max,
                )
                nc.vector.scalar_tensor_tensor(
                    out=dst, in0=e, scalar=1.0, in1=r,
                    op0=ALU.min, op1=ALU.add,
                )

            with (
                tc.tile_pool(name="x8", bufs=1) as xt8pool,
                tc.tile_pool(name="w8", bufs=2) as w8pool,
                tc.tile_pool(name="elu", bufs=2) as elupool,
            ):
                xt8_all = xt8pool.tile([128, KT, T], FP8)
                wk8_t = w8pool.tile([128, KT, H], FP8, tag="w8", name="wk8")
                wq8_t = w8pool.tile([128, KT, H], FP8, tag="w8", name="wq8")

                # staged initial loads, wide rearranged descriptors: tile 0
                # can start after ~1 MB (xt8+wk8 column halves); phase-2
                # inputs follow on the Sync queue.
                def xt8_load(c0, c1):
                    nc.sync.dma_start(
                        out=xt8_all[:, :, c0:c1],
                        in_=xt8_d[:, c0:c1].rearrange("(k p) c -> p k c",
                                                      p=128))

                def w8_load(t, src, c0, c1):
                    nc.scalar.dma_start(
                        out=t[:, :, c0:c1],
                        in_=src[:, c0:c1].rearrange("(k p) n -> p k n",
                                                    p=128))

                CH = min(512, T)
                xt8_load(0, min(256, CH))
                w8_load(wk8_t, wk8_d, 0, 512)
                if CH > 256:
                    xt8_load(256, CH)
                w8_load(wk8_t, wk8_d, 512, 1024)
                for h in range(1, T // CH):
                    xt8_load(CH * h, CH * (h + 1))
                w8_load(wq8_t, wq8_d, 0, 1024)
                for h in range(T // CH):
                    csl = slice(CH * h, CH * (h + 1))
                    nc.sync.dma_start(
                        out=xt_all[:, :, csl],
                        in_=xt_d[:, csl].rearrange("(k p) c -> p k c", p=128))
                load_w(wv_t, "wv")
                load_w(wg_t, "wg")

                def dr_proj(pk, w8_t, t):
                    # contraction 1024 as 4 DoubleRow pair-slices of 256
                    for n in range(2):
                        nsl = slice(512 * n, 512 * (n + 1))
                        for s in range(KP):
                            nc.tensor.matmul(
                                pk[:, nsl],
                                lhsT=xt8_all[:, 2 * s:2 * s + 2,
                                             128 * t:128 * (t + 1)],
                                rhs=w8_t[:, 2 * s:2 * s + 2, nsl],
                                start=(s == 0), stop=(s == KP - 1),
                                perf_mode=DR,
                            )

                # ---- phase 1a: k projection + k_sum (k kept in SBUF) ------
                # pk bufs=2: the 2 banks left free (after ks closes) let
                # 1b's pq pool start on banks that don't wait for 1a's
                # last elu to release pk.
                with (
                    tc.tile_pool(name="ks", bufs=1, space="PSUM") as kspool,
                    tc.tile_pool(name="pk", bufs=2, space="PSUM") as pkpool,
                ):
                    ks_ps = kspool.tile([1, H], F32)

                    def emit_ksum(t):
                        for n in range(2):
                            nc.tensor.matmul(
                                ks_ps[:, 512 * n:512 * (n + 1)],
                                lhsT=ones_col,
                                rhs=kt_all[:, t, 512 * n:512 * (n + 1)],
                                start=(t == 0),
                                stop=(t == NT - 1),
                            )

                    for t in range(NT):
                        pk = pkpool.tile([128, H], F32, tag="pk")
                        dr_proj(pk, wk8_t, t)
                        elu1(kt_all[:, t, :], pk)
                        # ksum of the previous tile: its elu chain finished
                        # while this tile's matmuls ran -> PE never waits
                        if t > 0:
                            emit_ksum(t - 1)
                    emit_ksum(NT - 1)
                    with tc.high_priority():
                        ks_sb = singles.tile([1, H], F32)
                        nc.vector.tensor_copy(out=ks_sb, in_=ks_ps)
                # real AR chain, all on the gpsimd queue (the Sync queue
                # still drains phase-2 bulk loads); ksb stays f32 -- one
                # broadcast DMA, no convert hop (costs +0.5us/tile on the
                # phase-2 nprod mul, which has headroom).
                with tc.high_priority():
                    nc.gpsimd.dma_start(out=ks_in[:, :], in_=ks_sb)
                    nc.gpsimd.collective_compute(
                        "AllReduce", ALU.add,
                        replica_groups=[list(g) for g in groups],
                        ins=[ks_in[:, :]], outs=[ks_out[:, :]],
                    )
                    ksb = singles.tile([128, H], F32)
                    nc.gpsimd.dma_start(
                        out=ksb, in_=ks_out[0:1, :].to_broadcast([128, H]))
                if apply_beta:
                    beta_b = singles.tile([128, H], BF16)
                    nc.gpsimd.dma_start(
                        out=beta_b, in_=beta_d[0:1, :].to_broadcast([128, H]))

                # ---- phase 1b: q projection (q kept in SBUF; qk deferred) --
                with tc.tile_pool(name="pq", bufs=2, space="PSUM") as pqpool:
                    for t in range(NT):
                        pq = pqpool.tile([128, H], F32, tag="pq")
                        dr_proj(pq, wq8_t, t)
                        elu1(qt_all[:, t, :], pq)

                # zero gate derived from 1b's last tile: phase-2 scalar ops
                # take it as bias so the scheduler cannot interleave them
                # into phase 1 (exp<->silu table thrash).
                gate0 = singles.tile([128, 1], F32)
                nc.vector.tensor_scalar(
                    out=gate0, in0=qt_all[:, NT - 1, 0:1],
                    scalar1=0.0, scalar2=None, op0=ALU.mult,
                )

            # ---------------- phase 2: v, g, z, LN, gate, Wo ----------------
            with ExitStack() as es2:
                pool2 = lambda n, b, **kw: es2.enter_context(
                    tc.tile_pool(name=n, bufs=b, **kw))
                wopool = pool2("wo", 1)
                vpool = pool2("vr", VBUFS)
                prodpool = pool2("prod", 1)
                zpool = pool2("z2", 3)
                spool = pool2("s2", PIPE + 1)
                upool = pool2("u2", PIPE + 1)
                utpool = pool2("ut", 3)
                ypool = pool2("y", 2)
                # rsig is consumed by back_end PIPE tiles later
                rspool = pool2("rs", PIPE + 1)
                # Wo + the v ring land in the bytes freed by xt8/wq8/wk8;
                # issued from the gpsimd queue (idle after the AR) because
                # the space frees only when 1b's last matmul retires -- a
                # blocked load on the scalar queue would deadlock behind the
                # gated silu.
                wo_t = wopool.tile([128, KT, H], BF16)
                nc.gpsimd.dma_start(
                    out=wo_t[:, :, :],
                    in_=w_d["wo"][:, :].rearrange("(k p) n -> p k n", p=128))
                # pool creation order controls PSUM bank placement: pa/pb
                # (needed at the first phase-2 matmul) grab the 4 banks that
                # were free during 1b; py (first needed ~35us in, PIPE deep)
                # takes the banks recycled from 1b's pq pool.
                if True:
                    papool = pool2("pa", 3, space="PSUM")
                    pbpool = pool2("pb", 3, space="PSUM")
                    pypool = pool2("py", 2, space="PSUM")

                    def back_end(u, rsig, t):
                        # u^T via the DMA XBAR hardware transpose (2-byte
                        # dtypes only); y = py * 1/sigma evacuates on ACT
                        # (Identity is in every table set -- no reload).
                        ut = utpool.tile([128, KT, 128], BF16, tag="ut")
                        nc.sync.dma_start_transpose(ut, u)
                        for n in range(2):
                            nsl = slice(512 * n, 512 * (n + 1))
                            py = pypool.tile([128, 512], F32, tag="py")
                            for k in range(KT):
                                mm(py, ut[:, k, :],
                                   wo_t[:, k, nsl], k == 0, k == KT - 1)
                            y_sb = ypool.tile([128, 512], BF16, tag="y")
                            if rsig is not None:
                                nc.scalar.activation(out=y_sb, in_=py,
                                                     func=ACT_F.Identity,
                                                     scale=rsig)
                            else:
                                nc.scalar.activation(out=y_sb, in_=py,
                                                     func=ACT_F.Identity)
                            nc.sync.dma_start(
                                out=out_d[128 * t:128 * (t + 1), nsl],
                                in_=y_sb)

                    # PIPE-deep software pipeline: tile t's back_end (wo
                    # matmuls) is enqueued at tile t+PIPE, giving the AR +
                    # ksb-gated DVE chain ~34us of PE runway at phase-2 start
                    prevs = []
                    for t in range(NT):
                        s_t = spool.tile([128, H], BF16, tag="s")
                        v_sb = vpool.tile([128, H], BF16, tag="v")
                        for n in range(2):
                            pv = papool.tile([128, 512], F32, tag="pa")
                            pg = pbpool.tile([128, 512], F32, tag="pb")
                            nsl = slice(512 * n, 512 * (n + 1))
                            for k in range(KT):
                                lhs = xt_all[:, k, 128 * t:128 * (t + 1)]
                                mm(pv, lhs, wv_t[:, k, nsl], k == 0, k == KT - 1)
                                mm(pg, lhs, wg_t[:, k, nsl], k == 0, k == KT - 1)
                            ssl = s_t[:, nsl]
                            if use_silu:
                                nc.scalar.activation(out=ssl, in_=pg,
                                                     func=ACT_F.Silu,
                                                     bias=gate0[:, 0:1])
                            else:  # CoreSim has no Silu table
                                nc.scalar.activation(out=ssl, in_=pg,
                                                     func=ACT_F.Sigmoid,
                                                     bias=gate0[:, 0:1])
                                nc.vector.tensor_mul(ssl, ssl, pg)
                            # v PSUM -> SBUF ring on ACT: frees pa so the
                            # v/g matmuls never wait on the ksb-gated DVE
                            nc.scalar.activation(out=v_sb[:, nsl], in_=pv,
                                                 func=ACT_F.Identity,
                                                 bias=gate0[:, 0:1])
                        # qk = per-head dot(q, k) -- deferred from 1b
                        prod = prodpool.tile([128, H], BF16, tag="prod")
                        nc.vector.tensor_mul(prod, qt_all[:, t, :],
                                             kt_all[:, t, :])
                        qk_t = smpool.tile([128, NH], F32, tag="qk")
                        nc.vector.reduce_sum(
                            out=qk_t,
                            in_=prod.rearrange("p (h d) -> p h d", d=DK),
                            axis=AX.X,
                        )
                        # normalizer = per-head dot(q, k_sum)
                        nprod = prodpool.tile([128, H], BF16, tag="prod")
                        nc.vector.tensor_mul(nprod, qt_all[:, t, :], ksb)
                        norm = smpool.tile([128, NH], F32, tag="norm")
                        nc.vector.reduce_sum(
                            out=norm,
                            in_=nprod.rearrange("p (h d) -> p h d", d=DK),
                            axis=AX.X,
                        )
                        rec = smpool.tile([128, NH], F32, tag="rec")
                        nc.vector.tensor_scalar_add(out=rec, in0=norm,
                                                    scalar1=1e-6)
                        nc.vector.reciprocal(out=rec, in_=rec)
                        r = smpool.tile([128, NH], F32, tag="r")
                        nc.vector.tensor_mul(r, qk_t, rec)
                        # z = r (broadcast over d) * v
                        z = zpool.tile([128, H], BF16, tag="z")
                        for n in range(2):
                            rs = r[:, 8 * n:8 * (n + 1)]
                            r_b = bass.AP(tensor=rs.tensor, offset=rs.offset,
                                          ap=[list(rs.ap[0]), list(rs.ap[1]),
                                              [0, DK]])
                            nc.vector.tensor_tensor(
                                out=z[:, 512 * n:512 * (n + 1)],
                                in0=v_sb[:, 512 * n:512 * (n + 1)],
                                in1=r_b, op=ALU.mult,
                            )
                        # LayerNorm stats over the full 1024 features
                        st = smpool.tile([128, 2, nc.vector.BN_STATS_DIM], F32,
                                         tag="bnst")
                        for n in range(2):
                            nc.vector.bn_stats(out=st[:, n, :],
                                               in_=z[:, 512 * n:512 * (n + 1)])
                        mv = smpool.tile([128, nc.vector.BN_AGGR_DIM], F32,
                                         tag="mv")
                        nc.vector.bn_aggr(out=mv, in_=st)
                        # rsig = rsqrt(var + eps) on the DVE: exponent
                        # bit-hack seed + 1 Newton step (rel err ~1.7e-3,
                        # ~2e-4 on the output).  Off the critical path;
                        # consumed only at Wo PSUM evacuation.
                        vq = smpool.tile([128, 1], F32, tag="vq")
                        nc.vector.tensor_scalar_add(out=vq, in0=mv[:, 1:2],
                                                    scalar1=1e-5)
                        rsig = rspool.tile([128, 1], F32, tag="rsig")
                        nc.vector.tensor_tensor(
                            out=rsig.bitcast(U32), in0=vq.bitcast(U32),
                            in1=c_shift1, op=ALU.logical_shift_right,
                        )
                        nc.vector.tensor_tensor(
                            out=rsig.bitcast(U32), in0=c_magic,
                            in1=rsig.bitcast(U32), op=ALU.subtract,
                        )
                        nt1 = smpool.tile([128, 1], F32, tag="nt1")
                        nc.vector.tensor_mul(nt1, rsig, rsig)
                        nc.vector.tensor_mul(nt1, nt1, vq)
                        nc.vector.tensor_scalar(
                            out=nt1, in0=nt1, scalar1=-0.5, scalar2=1.5,
                            op0=ALU.mult, op1=ALU.add,
                        )
                        nc.vector.tensor_mul(rsig, rsig, nt1)
                        # u = (z - mu) * silu(g) in one stt; 1/sigma deferred
                        u = upool.tile([128, H], BF16, tag="u")
                        if apply_beta:
                            # beta breaks the deferral: apply rsig here
                            nc.vector.tensor_scalar(
                                out=u, in0=z, scalar1=mv[:, 0:1], scalar2=rsig,
                                op0=ALU.subtract, op1=ALU.mult,
                            )
                            nc.vector.tensor_add(out=u, in0=u, in1=beta_b)
                            nc.vector.tensor_mul(u, u, s_t)
                            rsig_eff = None
                        else:
                            nc.vector.scalar_tensor_tensor(
                                out=u, in0=z, scalar=mv[:, 0:1], in1=s_t,
                                op0=ALU.subtract, op1=ALU.mult,
                            )
                            rsig_eff = rsig
                        prevs.append((u, rsig_eff, t))
                        # full depth only while the AR needs runway; ramp
                        # down near the end so the tail doesn't bunch
                        depth = PIPE if t < NT - 2 else 2
                        while len(prevs) > depth:
                            back_end(*prevs.pop(0))
                    for p in prevs:
                        back_end(*p)
    return _split_multi_waits(nc) if split_waits else nc


# ------------------------------------------------------------------
# host glue
# ------------------------------------------------------------------
_CACHE = {}
LAST_RESULT = None


def kernel(hidden_states, Wq, Wk, Wv, Wg, Wo, gamma, beta):
    import ml_dtypes
    bf16 = ml_dtypes.bfloat16
    e4m3 = ml_dtypes.float8_e4m3

    hs = np.asarray(hidden_states, dtype=np.float32)
    Wq = np.asarray(Wq, dtype=np.float32)
    Wk = np.asarray(Wk, dtype=np.float32)
    Wv = np.asarray(Wv, dtype=np.float32)
    Wg = np.asarray(Wg, dtype=np.float32)
    Wo = np.asarray(Wo, dtype=np.float32)
    gamma = np.asarray(gamma, dtype=np.float32)
    beta = np.asarray(beta, dtype=np.float32)

    b, s, h = hs.shape
    tokens = hs.reshape(b * s, h)
    n_tok = b * s
    T = n_tok // N_CORES
    assert s % T == 0, "core token shards must not straddle batches"
    cores_per_batch = s // T

    groups = tuple(
        tuple(range(bi * cores_per_batch, (bi + 1) * cores_per_batch))
        for bi in range(b)
    )
    apply_beta = bool(np.any(beta))

    key = (T, groups, apply_beta)
    if key not in _CACHE:
        _CACHE[key] = build_gla(T=T, groups=groups, apply_beta=apply_beta)
    nc = _CACHE[key]

    wo_eff = (gamma[:, None] * Wo).astype(bf16)
    wq8 = (Wq * WSCALE).astype(e4m3)
    wk8 = (Wk * WSCALE).astype(e4m3)
    wv_b = Wv.astype(bf16)
    wg_b = Wg.astype(bf16)
    in_maps = []
    for i in range(N_CORES):
        xt_f32 = np.ascontiguousarray(tokens[i * T:(i + 1) * T].T)
        m = {
            "xt": xt_f32.astype(bf16),
            "xt8": xt_f32.astype(e4m3),
            "wq8": wq8, "wk8": wk8,
            "wv": wv_b, "wg": wg_b, "wo": wo_eff,
        }
        if apply_beta:
            m["beta"] = beta.reshape(1, h).astype(bf16)
        in_maps.append(m)

    res = run_bass_kernel_spmd(
        nc, in_maps, core_ids=list(range(N_CORES)),
        trace=bool(os.environ.get("GLA_TRACE")),
    )
    global LAST_RESULT
    LAST_RESULT = res
    out = np.concatenate(
        [res.results[i]["out"].astype(np.float32) for i in range(N_CORES)],
        axis=0)
    return out.reshape(b, s, h)


# revision 24
# speedup vs baseline: 1.0189x; 1.0189x over previous
"""Gated linear attention kernel for one TRN2 chip (8 NeuronCores).

Math (see reference):
    q = elu(X Wq)+1, k = elu(X Wk)+1, v = X Wv, g = X Wg
    qk = sum_d(q*k) per head; k_sum = sum_seq(k); norm = sum_d(q*k_sum)
    z = qk*v/(norm+1e-6); z = LayerNorm(z)*gamma+beta; out = (z*silu(g)) Wo

Sharding: data-parallel over the 16384 tokens, 2048 per core; cores 0-3 own
batch 0, cores 4-7 batch 1.  The only cross-core coupling is k_sum (a [1,1024]
vector per batch) -> AllReduce within 4-core groups.

The kernel is PE-streaming-bound at the GPIO-throttled 1.95 GHz clock (the
13/16 clock-gate engages ~60us in; MMs pipeline at 263ns/512cols), so v2/v3
cut PE cycles and then keep every other engine strictly under the PE:
  * q/k projections in fp8-e4m3 perf_mode=DoubleRow: 2 fp8 MACs/cell/cycle,
    pairing adjacent 128-row k-slices via 3D APs [128,2,M]/[128,2,N]
    (HW-verified, 8e-4).  Host pre-scales Wq/Wk by 32; the 1/32 descale folds
    into the elu ACTs' scale operand.  End-to-end rel err 6.9e-3 (gate 2e-2);
    the q-quantization error cancels between qk and norm, and k_sum averages
    8192 positive terms.  v/g/o cannot go fp8 (3.5e-2+ each, and u underflows
    e4m3 by ~2^-13).
  * elu = min(exp(x),1)+relu(x) on [128,1024] 2-bank PSUM tiles; exp + half
    the relu on ACT (1.87us/tile), other relu half + combine on DVE
    (1.87us/tile), both under the 2.1us/tile of DR matmuls.
  * qk = sum_d(q*k) deferred to phase 2 where the DVE has slack.
  * The AllReduce takes ~45us trigger-to-done on this stack (mesh latency +
    skew + a one-time ~50us replica-group barrier).  A dummy warm-up
    AllReduce issued at kernel start absorbs the barrier; the real AR chain
    (vector hp copy -> gpsimd dma -> AR -> vector hp f32->bf16 -> gpsimd
    broadcast) fires right after 1a.  Phase 2 gives it ~34us of runway: the
    software pipeline is 4 deep and v PSUM evacuates to a 6-tile rolling
    SBUF buffer via scalar Identity, so v/g matmuls never wait on the
    ksb-gated DVE chain.
  * Phase-2 scalar ops (silu + v-evac) are hard-gated on a zero-bias AP
    derived from 1b's last tile: without it the scheduler interleaves silu
    into phase 1 and thrashes the exp<->silu ACT tables (8x1.3us reloads on
    the bottleneck engine).  Identity/relu live in every table set.
  * y = py*rsig evacuates on ACT (Identity, scale=rsig); u = (z-mu)*s in one
    stt; rsqrt uses 1 Newton step (rel err 1.7e-3 on rsig, ~0.2e-3 on out).
  * SBUF lifetimes: xt8/wq8/wk8 close after 1b; Wo + the v ring live in the
    freed bytes (Wo loads via the gpsimd queue -- a blocked load on the
    scalar queue would deadlock behind the gated silu).
  * Output is stored bf16 (halves write traffic), upcast on the host.
Carried over from v1: X^T/k/q SBUF-resident, DVE rsqrt Newton with 1/sigma
folded past Wo, DMA-XBAR transpose for u^T, k-sliced initial loads, gamma
folded into Wo on the host; beta==0 verified on the host.
"""

import os
from contextlib import ExitStack

import numpy as np

import concourse.bass as bass
import concourse.mybir as mybir
import concourse.tile as tile
from concourse.bass_utils import run_bass_kernel_spmd

F32 = mybir.dt.float32
BF16 = mybir.dt.bfloat16
FP8 = mybir.dt.float8e4
U32 = mybir.dt.uint32
AX = mybir.AxisListType
ALU = mybir.AluOpType
ACT_F = mybir.ActivationFunctionType
DR = mybir.MatmulPerfMode.DoubleRow

H = 1024
NH = 16
DK = 64
N_CORES = 8
WSCALE = 32.0          # host multiplies Wq/Wk by this before e4m3 quantization
ISCALE = 1.0 / WSCALE  # folded into the elu ACTs
PIPE = 4               # phase-2 software pipeline depth (AR runway)
VBUFS = 6              # rolling v ring tiles


def _split_multi_waits(nc, cap=1):
    """walrus in this image rejects instructions with more than ~2 sync waits
    (Tile attaches several to its kernel-tail drain).  Move excess waits onto
    preceding same-engine NoOps."""
    for f in nc.m.functions:
        for bb in f.blocks:
            insts = bb.instructions
            new_list = []
            changed = False
            for inst in insts:
                si = inst.sync_info
                waits = list(si.on_wait) if si else []
                if len(waits) > cap:
                    changed = True
                    for kk, w in enumerate(waits[:-cap]):
                        new_list.append(
                            mybir.InstNoOp(
                                name=f"{inst.name}-wsplit{kk}",
                                engine=inst.engine,
                                ins=[],
                                outs=[],
                                sync_info=mybir.SyncInfo(on_wait=[w], on_update=[]),
                            )
                        )
                    inst.sync_info = mybir.SyncInfo(
                        on_wait=waits[-cap:], on_update=list(si.on_update)
                    )
                new_list.append(inst)
            if changed:
                live = bb.instructions
                live.clear()
                for i in new_list:
                    bb.add_instruction(i)
    return nc


def build_gla(T=2048, groups=((0, 1, 2, 3), (4, 5, 6, 7)), n_devices=8,
              apply_beta=False, split_waits=True, use_silu=True):
    """Build the per-core SPMD program.  T = tokens per core."""
    assert T % 128 == 0
    NT = T // 128      # 128-token tiles
    KT = H // 128      # contraction slices
    KP = KT // 2       # DoubleRow k-pair slices

    nc = bass.Bass(num_devices=n_devices)
    xt_d = nc.declare_dram_parameter("xt", [H, T], BF16, isOutput=False)
    xt8_d = nc.declare_dram_parameter("xt8", [H, T], FP8, isOutput=False)
    wq8_d = nc.declare_dram_parameter("wq8", [H, H], FP8, isOutput=False)
    wk8_d = nc.declare_dram_parameter("wk8", [H, H], FP8, isOutput=False)
    w_d = {
        n: nc.declare_dram_parameter(n, [H, H], BF16, isOutput=False)
        for n in ("wv", "wg", "wo")
    }
    beta_d = (
        nc.declare_dram_parameter("beta", [1, H], BF16, isOutput=False)
        if apply_beta
        else None
    )
    out_d = nc.declare_dram_parameter("out", [T, H], BF16, isOutput=True)

    ks_in = nc.dram_tensor("ks_in", [1, H], F32)
    ks_out = nc.dram_tensor("ks_out", [1, H], F32)
    arw_in = nc.dram_tensor("arw_in", [1, 8], F32)
    arw_out = nc.dram_tensor("arw_out", [1, 8], F32)

    def mm(ps, lhsT, rhs, start, stop):
        nc.tensor.matmul(ps, lhsT=lhsT, rhs=rhs, start=start, stop=stop)

    with tile.TileContext(nc) as tc:
        with (
            tc.tile_pool(name="singles", bufs=1) as singles,
            tc.tile_pool(name="w", bufs=2) as wpool,
            tc.tile_pool(name="xt", bufs=1) as xtpool,
            tc.tile_pool(name="kt", bufs=1) as ktpool,
            tc.tile_pool(name="qt", bufs=1) as qtpool,
            tc.tile_pool(name="small", bufs=3) as smpool,
        ):
            ones_col = singles.tile([128, 1], BF16)
            nc.vector.memset(ones_col, 1.0)
            # rsqrt bit-hack constants (as APs: immediate ints on uint ops
            # are unreliable through the f32 immediate path)
            c_shift1 = singles.tile([128, 1], U32)
            nc.vector.memset(c_shift1, 1)
            c_magic = singles.tile([128, 1], U32)
            nc.vector.memset(c_magic, 0x5F3759DF)

            # warm-up AllReduce: establishes the replica-group barrier +
            # CC stream (~50us, one-time) while phase 1 computes, so the
            # real k_sum AR only pays ring latency.
            arw_sb = singles.tile([1, 8], F32)
            nc.gpsimd.memset(arw_sb, 0.0)
            nc.gpsimd.dma_start(out=arw_in[:, :], in_=arw_sb)
            nc.gpsimd.collective_compute(
                "AllReduce", ALU.add,
                replica_groups=[list(g) for g in groups],
                ins=[arw_in[:, :]], outs=[arw_out[:, :]],
            )

            xt_all = xtpool.tile([128, KT, T], BF16)
            kt_all = ktpool.tile([128, NT, H], BF16)
            qt_all = qtpool.tile([128, NT, H], BF16)
            wv_t = wpool.tile([128, KT, H], BF16, tag="w", name="wv")
            wg_t = wpool.tile([128, KT, H], BF16, tag="w", name="wg")

            def load_w(t, name, engine=None):
                # one wide descriptor: [H, H] viewed as [p, k-slice, cols]
                (engine or nc.sync).dma_start(
                    out=t[:, :, :],
                    in_=w_d[name][:, :].rearrange("(k p) n -> p k n", p=128))

            def elu1(dst, ps):
                # dst = elu(ps/32)+1 = min(exp(ps/32), 1) + relu(ps/32);
                # exp + low relu half on ACT, high relu half + combine on DVE
                e = elupool.tile([128, H], BF16, tag="elue")
                r = elupool.tile([128, H], BF16, tag="elur")
                nc.scalar.activation(out=e, in_=ps, func=ACT_F.Exp,
                                     scale=ISCALE)
                nc.scalar.activation(out=r[:, 0:512], in_=ps[:, 0:512],
                                     func=ACT_F.Relu, scale=ISCALE)
                nc.vector.tensor_scalar(
                    out=r[:, 512:H], in0=ps[:, 512:H],
                    scalar1=ISCALE, scalar2=0.0, op0=ALU.mult, op1=ALU.max,
                )
                nc.vector.scalar_tensor_tensor(
                    out=dst, in0=e, scalar=1.0, in1=r,
                    op0=ALU.min, op1=ALU.add,
                )

            with (
                tc.tile_pool(name="x8", bufs=1) as xt8pool,
                tc.tile_pool(name="w8", bufs=2) as w8pool,
                tc.tile_pool(name="elu", bufs=2) as elupool,
            ):
                xt8_all = xt8pool.tile([128, KT, T], FP8)
                wk8_t = w8pool.tile([128, KT, H], FP8, tag="w8", name="wk8")
                wq8_t = w8pool.tile([128, KT, H], FP8, tag="w8", name="wq8")

                # staged initial loads, wide rearranged descriptors: tile 0
                # can start after ~1 MB (xt8+wk8 column halves); phase-2
                # inputs follow on the Sync queue.
                def xt8_load(c0, c1):
                    nc.sync.dma_start(
                        out=xt8_all[:, :, c0:c1],
                        in_=xt8_d[:, c0:c1].rearrange("(k p) c -> p k c",
                                                      p=128))

                def w8_load(t, src, c0, c1):
                    nc.scalar.dma_start(
                        out=t[:, :, c0:c1],
                        in_=src[:, c0:c1].rearrange("(k p) n -> p k n",
                                                    p=128))

                CH = min(512, T)
                xt8_load(0, CH)
                w8_load(wk8_t, wk8_d, 0, 512)
                w8_load(wk8_t, wk8_d, 512, 1024)
                for h in range(1, T // CH):
                    xt8_load(CH * h, CH * (h + 1))
                w8_load(wq8_t, wq8_d, 0, 1024)
                for h in range(T // CH):
                    csl = slice(CH * h, CH * (h + 1))
                    nc.sync.dma_start(
                        out=xt_all[:, :, csl],
                        in_=xt_d[:, csl].rearrange("(k p) c -> p k c", p=128))
                load_w(wv_t, "wv")
                load_w(wg_t, "wg")

                def dr_proj(pk, w8_t, t):
                    # contraction 1024 as 4 DoubleRow pair-slices of 256
                    for n in range(2):
                        nsl = slice(512 * n, 512 * (n + 1))
                        for s in range(KP):
                            nc.tensor.matmul(
                                pk[:, nsl],
                                lhsT=xt8_all[:, 2 * s:2 * s + 2,
                                             128 * t:128 * (t + 1)],
                                rhs=w8_t[:, 2 * s:2 * s + 2, nsl],
                                start=(s == 0), stop=(s == KP - 1),
                                perf_mode=DR,
                            )

                # ---- phase 1a: k projection + k_sum (k kept in SBUF) ------
                with (
                    tc.tile_pool(name="ks", bufs=1, space="PSUM") as kspool,
                    tc.tile_pool(name="pk", bufs=3, space="PSUM") as pkpool,
                ):
                    ks_ps = kspool.tile([1, H], F32)

                    def emit_ksum(t):
                        for n in range(2):
                            nc.tensor.matmul(
                                ks_ps[:, 512 * n:512 * (n + 1)],
                                lhsT=ones_col,
                                rhs=kt_all[:, t, 512 * n:512 * (n + 1)],
                                start=(t == 0),
                                stop=(t == NT - 1),
                            )

                    for t in range(NT):
                        pk = pkpool.tile([128, H], F32, tag="pk")
                        dr_proj(pk, wk8_t, t)
                        elu1(kt_all[:, t, :], pk)
                        # ksum of the previous tile: its elu chain finished
                        # while this tile's matmuls ran -> PE never waits
                        if t > 0:
                            emit_ksum(t - 1)
                    emit_ksum(NT - 1)
                    with tc.high_priority():
                        ks_sb = singles.tile([1, H], F32)
                        nc.vector.tensor_copy(out=ks_sb, in_=ks_ps)
                # real AR chain, all on the gpsimd queue (the Sync queue
                # still drains phase-2 bulk loads); ksb stays f32 -- one
                # broadcast DMA, no convert hop (costs +0.5us/tile on the
                # phase-2 nprod mul, which has headroom).
                with tc.high_priority():
                    nc.gpsimd.dma_start(out=ks_in[:, :], in_=ks_sb)
                    nc.gpsimd.collective_compute(
                        "AllReduce", ALU.add,
                        replica_groups=[list(g) for g in groups],
                        ins=[ks_in[:, :]], outs=[ks_out[:, :]],
                    )
                    ksb = singles.tile([128, H], F32)
                    nc.gpsimd.dma_start(
                        out=ksb, in_=ks_out[0:1, :].to_broadcast([128, H]))
                if apply_beta:
                    beta_b = singles.tile([128, H], BF16)
                    nc.gpsimd.dma_start(
                        out=beta_b, in_=beta_d[0:1, :].to_broadcast([128, H]))

                # ---- phase 1b: q projection (q kept in SBUF; qk deferred) --
                with tc.tile_pool(name="pq", bufs=2, space="PSUM") as pqpool:
                    for t in range(NT):
                        pq = pqpool.tile([128, H], F32, tag="pq")
                        dr_proj(pq, wq8_t, t)
                        elu1(qt_all[:, t, :], pq)

                # zero gate derived from 1b's last tile: phase-2 scalar ops
                # take it as bias so the scheduler cannot interleave them
                # into phase 1 (exp<->silu table thrash).
                gate0 = singles.tile([128, 1], F32)
                nc.vector.tensor_scalar(
                    out=gate0, in0=qt_all[:, NT - 1, 0:1],
                    scalar1=0.0, scalar2=None, op0=ALU.mult,
                )

            # ---------------- phase 2: v, g, z, LN, gate, Wo ----------------
            with ExitStack() as es2:
                pool2 = lambda n, b, **kw: es2.enter_context(
                    tc.tile_pool(name=n, bufs=b, **kw))
                wopool = pool2("wo", 1)
                vpool = pool2("vr", VBUFS)
                prodpool = pool2("prod", 1)
                zpool = pool2("z2", 3)
                spool = pool2("s2", PIPE + 1)
                upool = pool2("u2", PIPE + 1)
                utpool = pool2("ut", 3)
                ypool = pool2("y", 2)
                # rsig is consumed by back_end PIPE tiles later
                rspool = pool2("rs", PIPE + 1)
                # Wo + the v ring land in the bytes freed by xt8/wq8/wk8;
                # issued from the gpsimd queue (idle after the AR) because
                # the space frees only when 1b's last matmul retires -- a
                # blocked load on the scalar queue would deadlock behind the
                # gated silu.
                wo_t = wopool.tile([128, KT, H], BF16)
                nc.gpsimd.dma_start(
                    out=wo_t[:, :, :],
                    in_=w_d["wo"][:, :].rearrange("(k p) n -> p k n", p=128))
                # pool creation order controls PSUM bank placement: pa/pb
                # (needed at the first phase-2 matmul) grab the 4 banks that
                # were free during 1b; py (first needed ~35us in, PIPE deep)
                # takes the banks recycled from 1b's pq pool.
                if True:
                    papool = pool2("pa", 3, space="PSUM")
                    pbpool = pool2("pb", 3, space="PSUM")
                    pypool = pool2("py", 2, space="PSUM")

                    def back_end(u, rsig, t):
                        # u^T via the DMA XBAR hardware transpose (2-byte
                        # dtypes only); y = py * 1/sigma evacuates on ACT
                        # (Identity is in every table set -- no reload).
                        ut = utpool.tile([128, KT, 128], BF16, tag="ut")
                        nc.sync.dma_start_transpose(ut, u)
                        for n in range(2):
                            nsl = slice(512 * n, 512 * (n + 1))
                            py = pypool.tile([128, 512], F32, tag="py")
                            for k in range(KT):
                                mm(py, ut[:, k, :],
                                   wo_t[:, k, nsl], k == 0, k == KT - 1)
                            y_sb = ypool.tile([128, 512], BF16, tag="y")
                            if rsig is not None:
                                nc.scalar.activation(out=y_sb, in_=py,
                                                     func=ACT_F.Identity,
                                                     scale=rsig)
                            else:
                                nc.scalar.activation(out=y_sb, in_=py,
                                                     func=ACT_F.Identity)
                            nc.sync.dma_start(
                                out=out_d[128 * t:128 * (t + 1), nsl],
                                in_=y_sb)

                    # PIPE-deep software pipeline: tile t's back_end (wo
                    # matmuls) is enqueued at tile t+PIPE, giving the AR +
                    # ksb-gated DVE chain ~34us of PE runway at phase-2 start
                    prevs = []
                    for t in range(NT):
                        s_t = spool.tile([128, H], BF16, tag="s")
                        v_sb = vpool.tile([128, H], BF16, tag="v")
                        for n in range(2):
                            pv = papool.tile([128, 512], F32, tag="pa")
                            pg = pbpool.tile([128, 512], F32, tag="pb")
                            nsl = slice(512 * n, 512 * (n + 1))
                            for k in range(KT):
                                lhs = xt_all[:, k, 128 * t:128 * (t + 1)]
                                mm(pv, lhs, wv_t[:, k, nsl], k == 0, k == KT - 1)
                                mm(pg, lhs, wg_t[:, k, nsl], k == 0, k == KT - 1)
                            ssl = s_t[:, nsl]
                            if use_silu:
                                nc.scalar.activation(out=ssl, in_=pg,
                                                     func=ACT_F.Silu,
                                                     bias=gate0[:, 0:1])
                            else:  # CoreSim has no Silu table
                                nc.scalar.activation(out=ssl, in_=pg,
                                                     func=ACT_F.Sigmoid,
                                                     bias=gate0[:, 0:1])
                                nc.vector.tensor_mul(ssl, ssl, pg)
                            # v PSUM -> SBUF ring on ACT: frees pa so the
                            # v/g matmuls never wait on the ksb-gated DVE
                            nc.scalar.activation(out=v_sb[:, nsl], in_=pv,
                                                 func=ACT_F.Identity,
                                                 bias=gate0[:, 0:1])
                        # qk = per-head dot(q, k) -- deferred from 1b.  Both
                        # prod muls go through stt with the zero gate as the
                        # scalar: without it the scheduler front-runs them
                        # into 1b's DVE queue (they only need qt/kt tiles)
                        # and overloads it past the PE rate.
                        prod = prodpool.tile([128, H], BF16, tag="prod")
                        nc.vector.scalar_tensor_tensor(
                            out=prod, in0=qt_all[:, t, :], scalar=gate0[:, 0:1],
                            in1=kt_all[:, t, :], op0=ALU.add, op1=ALU.mult,
                        )
                        qk_t = smpool.tile([128, NH], F32, tag="qk")
                        nc.vector.reduce_sum(
                            out=qk_t,
                            in_=prod.rearrange("p (h d) -> p h d", d=DK),
                            axis=AX.X,
                        )
                        # normalizer = per-head dot(q, k_sum)
                        nprod = prodpool.tile([128, H], BF16, tag="prod")
                        nc.vector.scalar_tensor_tensor(
                            out=nprod, in0=qt_all[:, t, :], scalar=gate0[:, 0:1],
                            in1=ksb, op0=ALU.add, op1=ALU.mult,
                        )
                        norm = smpool.tile([128, NH], F32, tag="norm")
                        nc.vector.reduce_sum(
                            out=norm,
                            in_=nprod.rearrange("p (h d) -> p h d", d=DK),
                            axis=AX.X,
                        )
                        rec = smpool.tile([128, NH], F32, tag="rec")
                        nc.vector.tensor_scalar_add(out=rec, in0=norm,
                                                    scalar1=1e-6)
                        nc.vector.reciprocal(out=rec, in_=rec)
                        r = smpool.tile([128, NH], F32, tag="r")
                        nc.vector.tensor_mul(r, qk_t, rec)
                        # z = r (broadcast over d) * v
                        z = zpool.tile([128, H], BF16, tag="z")
                        for n in range(2):
                            rs = r[:, 8 * n:8 * (n + 1)]
                            r_b = bass.AP(tensor=rs.tensor, offset=rs.offset,
                                          ap=[list(rs.ap[0]), list(rs.ap[1]),
                                              [0, DK]])
                            nc.vector.tensor_tensor(
                                out=z[:, 512 * n:512 * (n + 1)],
                                in0=v_sb[:, 512 * n:512 * (n + 1)],
                                in1=r_b, op=ALU.mult,
                            )
                        # LayerNorm stats over the full 1024 features
                        st = smpool.tile([128, 2, nc.vector.BN_STATS_DIM], F32,
                                         tag="bnst")
                        for n in range(2):
                            nc.vector.bn_stats(out=st[:, n, :],
                                               in_=z[:, 512 * n:512 * (n + 1)])
                        mv = smpool.tile([128, nc.vector.BN_AGGR_DIM], F32,
                                         tag="mv")
                        nc.vector.bn_aggr(out=mv, in_=st)
                        # rsig = rsqrt(var + eps) on the DVE: exponent
                        # bit-hack seed + 1 Newton step (rel err ~1.7e-3,
                        # ~2e-4 on the output).  Off the critical path;
                        # consumed only at Wo PSUM evacuation.
                        vq = smpool.tile([128, 1], F32, tag="vq")
                        nc.vector.tensor_scalar_add(out=vq, in0=mv[:, 1:2],
                                                    scalar1=1e-5)
                        rsig = rspool.tile([128, 1], F32, tag="rsig")
                        nc.vector.tensor_tensor(
                            out=rsig.bitcast(U32), in0=vq.bitcast(U32),
                            in1=c_shift1, op=ALU.logical_shift_right,
                        )
                        nc.vector.tensor_tensor(
                            out=rsig.bitcast(U32), in0=c_magic,
                            in1=rsig.bitcast(U32), op=ALU.subtract,
                        )
                        nt1 = smpool.tile([128, 1], F32, tag="nt1")
                        nc.vector.tensor_mul(nt1, rsig, rsig)
                        nc.vector.tensor_mul(nt1, nt1, vq)
                        nc.vector.tensor_scalar(
                            out=nt1, in0=nt1, scalar1=-0.5, scalar2=1.5,
                            op0=ALU.mult, op1=ALU.add,
                        )
                        nc.vector.tensor_mul(rsig, rsig, nt1)
                        # u = (z - mu) * silu(g) in one stt; 1/sigma deferred
                        u = upool.tile([128, H], BF16, tag="u")
                        if apply_beta:
                            # beta breaks the deferral: apply rsig here
                            nc.vector.tensor_scalar(
                                out=u, in0=z, scalar1=mv[:, 0:1], scalar2=rsig,
                                op0=ALU.subtract, op1=ALU.mult,
                            )
                            nc.vector.tensor_add(out=u, in0=u, in1=beta_b)
                            nc.vector.tensor_mul(u, u, s_t)
                            rsig_eff = None
                        else:
                            nc.vector.scalar_tensor_tensor(
                                out=u, in0=z, scalar=mv[:, 0:1], in1=s_t,
                                op0=ALU.subtract, op1=ALU.mult,
                            )
                            rsig_eff = rsig
                        prevs.append((u, rsig_eff, t))
                        # full depth only while the AR needs runway; ramp
                        # down near the end so the tail doesn't bunch
                        depth = PIPE if t < NT - 2 else 2
                        while len(prevs) > depth:
                            back_end(*prevs.pop(0))
                    for p in prevs:
                        back_end(*p)
    return _split_multi_waits(nc) if split_waits else nc


# ------------------------------------------------------------------
# host glue
# ------------------------------------------------------------------
_CACHE = {}
LAST_RESULT = None


def kernel(hidden_states, Wq, Wk, Wv, Wg, Wo, gamma, beta):
    import ml_dtypes
    bf16 = ml_dtypes.bfloat16
    e4m3 = ml_dtypes.float8_e4m3

    hs = np.asarray(hidden_states, dtype=np.float32)
    Wq = np.asarray(Wq, dtype=np.float32)
    Wk = np.asarray(Wk, dtype=np.float32)
    Wv = np.asarray(Wv, dtype=np.float32)
    Wg = np.asarray(Wg, dtype=np.float32)
    Wo = np.asarray(Wo, dtype=np.float32)
    gamma = np.asarray(gamma, dtype=np.float32)
    beta = np.asarray(beta, dtype=np.float32)

    b, s, h = hs.shape
    tokens = hs.reshape(b * s, h)
    n_tok = b * s
    T = n_tok // N_CORES
    assert s % T == 0, "core token shards must not straddle batches"
    cores_per_batch = s // T

    groups = tuple(
        tuple(range(bi * cores_per_batch, (bi + 1) * cores_per_batch))
        for bi in range(b)
    )
    apply_beta = bool(np.any(beta))

    key = (T, groups, apply_beta)
    if key not in _CACHE:
        _CACHE[key] = build_gla(T=T, groups=groups, apply_beta=apply_beta)
    nc = _CACHE[key]

    wo_eff = (gamma[:, None] * Wo).astype(bf16)
    wq8 = (Wq * WSCALE).astype(e4m3)
    wk8 = (Wk * WSCALE).astype(e4m3)
    wv_b = Wv.astype(bf16)
    wg_b = Wg.astype(bf16)
    in_maps = []
    for i in range(N_CORES):
        xt_f32 = np.ascontiguousarray(tokens[i * T:(i + 1) * T].T)
        m = {
            "xt": xt_f32.astype(bf16),
            "xt8": xt_f32.astype(e4m3),
            "wq8": wq8, "wk8": wk8,
            "wv": wv_b, "wg": wg_b, "wo": wo_eff,
        }
        if apply_beta:
            m["beta"] = beta.reshape(1, h).astype(bf16)
        in_maps.append(m)

    res = run_bass_kernel_spmd(
        nc, in_maps, core_ids=list(range(N_CORES)),
        trace=bool(os.environ.get("GLA_TRACE")),
    )
    global LAST_RESULT
    LAST_RESULT = res
    out = np.concatenate(
        [res.results[i]["out"].astype(np.float32) for i in range(N_CORES)],
        axis=0)
    return out.reshape(b, s, h)


# revision 28
# speedup vs baseline: 1.0228x; 1.0038x over previous
"""Gated linear attention kernel for one TRN2 chip (8 NeuronCores).

Math (see reference):
    q = elu(X Wq)+1, k = elu(X Wk)+1, v = X Wv, g = X Wg
    qk = sum_d(q*k) per head; k_sum = sum_seq(k); norm = sum_d(q*k_sum)
    z = qk*v/(norm+1e-6); z = LayerNorm(z)*gamma+beta; out = (z*silu(g)) Wo

Sharding: data-parallel over the 16384 tokens, 2048 per core; cores 0-3 own
batch 0, cores 4-7 batch 1.  The only cross-core coupling is k_sum (a [1,1024]
vector per batch) -> AllReduce within 4-core groups.

The kernel is PE-streaming-bound at the GPIO-throttled 1.95 GHz clock (the
13/16 clock-gate engages ~60us in; MMs pipeline at 263ns/512cols), so v2/v3
cut PE cycles and then keep every other engine strictly under the PE:
  * q/k projections in fp8-e4m3 perf_mode=DoubleRow: 2 fp8 MACs/cell/cycle,
    pairing adjacent 128-row k-slices via 3D APs [128,2,M]/[128,2,N]
    (HW-verified, 8e-4).  Host pre-scales Wq/Wk by 32; the 1/32 descale folds
    into the elu ACTs' scale operand.  End-to-end rel err 6.9e-3 (gate 2e-2);
    the q-quantization error cancels between qk and norm, and k_sum averages
    8192 positive terms.  v/g/o cannot go fp8 (3.5e-2+ each, and u underflows
    e4m3 by ~2^-13).
  * elu = min(exp(x),1)+relu(x) on [128,1024] 2-bank PSUM tiles; exp + half
    the relu on ACT (1.87us/tile), other relu half + combine on DVE
    (1.87us/tile), both under the 2.1us/tile of DR matmuls.
  * qk = sum_d(q*k) deferred to phase 2 where the DVE has slack.
  * The AllReduce takes ~45us trigger-to-done on this stack (mesh latency +
    skew + a one-time ~50us replica-group barrier).  A dummy warm-up
    AllReduce issued at kernel start absorbs the barrier; the real AR chain
    (vector hp copy -> gpsimd dma -> AR -> vector hp f32->bf16 -> gpsimd
    broadcast) fires right after 1a.  Phase 2 gives it ~34us of runway: the
    software pipeline is 4 deep and v PSUM evacuates to a 6-tile rolling
    SBUF buffer via scalar Identity, so v/g matmuls never wait on the
    ksb-gated DVE chain.
  * Phase-2 scalar ops (silu + v-evac) are hard-gated on a zero-bias AP
    derived from 1b's last tile: without it the scheduler interleaves silu
    into phase 1 and thrashes the exp<->silu ACT tables (8x1.3us reloads on
    the bottleneck engine).  Identity/relu live in every table set.
  * y = py*rsig evacuates on ACT (Identity, scale=rsig); u = (z-mu)*s in one
    stt; rsqrt uses 1 Newton step (rel err 1.7e-3 on rsig, ~0.2e-3 on out).
  * SBUF lifetimes: xt8/wq8/wk8 close after 1b; Wo + the v ring live in the
    freed bytes (Wo loads via the gpsimd queue -- a blocked load on the
    scalar queue would deadlock behind the gated silu).
  * Output is stored bf16 (halves write traffic), upcast on the host.
Carried over from v1: X^T/k/q SBUF-resident, DVE rsqrt Newton with 1/sigma
folded past Wo, DMA-XBAR transpose for u^T, k-sliced initial loads, gamma
folded into Wo on the host; beta==0 verified on the host.
"""

import os
from contextlib import ExitStack

import numpy as np

import concourse.bass as bass
import concourse.mybir as mybir
import concourse.tile as tile
from concourse.bass_utils import run_bass_kernel_spmd

F32 = mybir.dt.float32
BF16 = mybir.dt.bfloat16
FP8 = mybir.dt.float8e4
U32 = mybir.dt.uint32
AX = mybir.AxisListType
ALU = mybir.AluOpType
ACT_F = mybir.ActivationFunctionType
DR = mybir.MatmulPerfMode.DoubleRow

H = 1024
NH = 16
DK = 64
N_CORES = 8
WSCALE = 32.0          # host multiplies Wq/Wk by this before e4m3 quantization
ISCALE = 1.0 / WSCALE  # folded into the elu ACTs
PIPE = 4               # phase-2 software pipeline depth (AR runway)
VBUFS = 6              # rolling v ring tiles


def _split_multi_waits(nc, cap=1):
    """walrus in this image rejects instructions with more than ~2 sync waits
    (Tile attaches several to its kernel-tail drain).  Move excess waits onto
    preceding same-engine NoOps."""
    for f in nc.m.functions:
        for bb in f.blocks:
            insts = bb.instructions
            new_list = []
            changed = False
            for inst in insts:
                si = inst.sync_info
                waits = list(si.on_wait) if si else []
                if len(waits) > cap:
                    changed = True
                    for kk, w in enumerate(waits[:-cap]):
                        new_list.append(
                            mybir.InstNoOp(
                                name=f"{inst.name}-wsplit{kk}",
                                engine=inst.engine,
                                ins=[],
                                outs=[],
                                sync_info=mybir.SyncInfo(on_wait=[w], on_update=[]),
                            )
                        )
                    inst.sync_info = mybir.SyncInfo(
                        on_wait=waits[-cap:], on_update=list(si.on_update)
                    )
                new_list.append(inst)
            if changed:
                live = bb.instructions
                live.clear()
                for i in new_list:
                    bb.add_instruction(i)
    return nc


def build_gla(T=2048, groups=((0, 1, 2, 3), (4, 5, 6, 7)), n_devices=8,
              apply_beta=False, split_waits=True, use_silu=True):
    """Build the per-core SPMD program.  T = tokens per core."""
    assert T % 128 == 0
    NT = T // 128      # 128-token tiles
    KT = H // 128      # contraction slices
    KP = KT // 2       # DoubleRow k-pair slices

    nc = bass.Bass(num_devices=n_devices)
    xt_d = nc.declare_dram_parameter("xt", [H, T], BF16, isOutput=False)
    xt8_d = nc.declare_dram_parameter("xt8", [H, T], FP8, isOutput=False)
    wq8_d = nc.declare_dram_parameter("wq8", [H, H], FP8, isOutput=False)
    wk8_d = nc.declare_dram_parameter("wk8", [H, H], FP8, isOutput=False)
    w_d = {
        n: nc.declare_dram_parameter(n, [H, H], BF16, isOutput=False)
        for n in ("wv", "wg", "wo")
    }
    beta_d = (
        nc.declare_dram_parameter("beta", [1, H], BF16, isOutput=False)
        if apply_beta
        else None
    )
    out_d = nc.declare_dram_parameter("out", [T, H], BF16, isOutput=True)

    ks_in = nc.dram_tensor("ks_in", [1, H], F32)
    ks_out = nc.dram_tensor("ks_out", [1, H], F32)
    arw_in = nc.dram_tensor("arw_in", [1, 8], F32)
    arw_out = nc.dram_tensor("arw_out", [1, 8], F32)

    def mm(ps, lhsT, rhs, start, stop):
        nc.tensor.matmul(ps, lhsT=lhsT, rhs=rhs, start=start, stop=stop)

    with tile.TileContext(nc) as tc:
        with (
            tc.tile_pool(name="singles", bufs=1) as singles,
            tc.tile_pool(name="w", bufs=2) as wpool,
            tc.tile_pool(name="xt", bufs=1) as xtpool,
            tc.tile_pool(name="kt", bufs=1) as ktpool,
            tc.tile_pool(name="qt", bufs=1) as qtpool,
            tc.tile_pool(name="small", bufs=3) as smpool,
        ):
            # fp8 ones pair for the DoubleRow k_sum (the 16-wide tile keeps
            # the pair-axis byte step at 16, a DR weight-AP constraint)
            ones8 = singles.tile([128, 2, 16], FP8)
            nc.vector.memset(ones8, 1.0)
            # rsqrt bit-hack constants (as APs: immediate ints on uint ops
            # are unreliable through the f32 immediate path)
            c_shift1 = singles.tile([128, 1], U32)
            nc.vector.memset(c_shift1, 1)
            c_magic = singles.tile([128, 1], U32)
            nc.vector.memset(c_magic, 0x5F3759DF)

            # warm-up AllReduce: establishes the replica-group barrier +
            # CC stream (~50us, one-time) while phase 1 computes, so the
            # real k_sum AR only pays ring latency.
            arw_sb = singles.tile([1, 8], F32)
            nc.gpsimd.memset(arw_sb, 0.0)
            nc.gpsimd.dma_start(out=arw_in[:, :], in_=arw_sb)
            nc.gpsimd.collective_compute(
                "AllReduce", ALU.add,
                replica_groups=[list(g) for g in groups],
                ins=[arw_in[:, :]], outs=[arw_out[:, :]],
            )

            # k is stored fp8: the elu stt writes e4m3 directly, k_sum runs
            # as DoubleRow tile-pairs (half the PE cycles), and the same
            # quantized k feeds qk AND (via k_sum) norm, so the error mostly
            # cancels in the ratio -- host-simulated rel err 8.2e-3.
            xt_all = xtpool.tile([128, KT, T], BF16)
            kt_all = ktpool.tile([128, NT, H], FP8)
            qt_all = qtpool.tile([128, NT, H], BF16)
            wv_t = wpool.tile([128, KT, H], BF16, tag="w", name="wv")
            wg_t = wpool.tile([128, KT, H], BF16, tag="w", name="wg")

            def load_w(t, name, engine=None):
                # one wide descriptor: [H, H] viewed as [p, k-slice, cols]
                (engine or nc.sync).dma_start(
                    out=t[:, :, :],
                    in_=w_d[name][:, :].rearrange("(k p) n -> p k n", p=128))

            def elu1(dst, ps):
                # dst = elu(ps/32)+1 = min(exp(ps/32), 1) + relu(ps/32);
                # exp + low relu half on ACT, high relu half + combine on DVE
                e = elupool.tile([128, H], BF16, tag="elue")
                r = elupool.tile([128, H], BF16, tag="elur")
                nc.scalar.activation(out=e, in_=ps, func=ACT_F.Exp,
                                     scale=ISCALE)
                nc.scalar.activation(out=r[:, 0:512], in_=ps[:, 0:512],
                                     func=ACT_F.Relu, scale=ISCALE)
                nc.vector.tensor_scalar(
                    out=r[:, 512:H], in0=ps[:, 512:H],
                    scalar1=ISCALE, scalar2=0.0, op0=ALU.mult, op1=ALU.max,
                )
                nc.vector.scalar_tensor_tensor(
                    out=dst, in0=e, scalar=1.0, in1=r,
                    op0=ALU.min, op1=ALU.add,
                )

            with (
                tc.tile_pool(name="x8", bufs=1) as xt8pool,
                tc.tile_pool(name="w8", bufs=2) as w8pool,
                tc.tile_pool(name="elu", bufs=2) as elupool,
            ):
                xt8_all = xt8pool.tile([128, KT, T], FP8)
                wk8_t = w8pool.tile([128, KT, H], FP8, tag="w8", name="wk8")
                wq8_t = w8pool.tile([128, KT, H], FP8, tag="w8", name="wq8")

                # staged initial loads, wide rearranged descriptors: tile 0
                # can start after ~1 MB (xt8+wk8 column halves); phase-2
                # inputs follow on the Sync queue.
                def xt8_load(c0, c1):
                    nc.sync.dma_start(
                        out=xt8_all[:, :, c0:c1],
                        in_=xt8_d[:, c0:c1].rearrange("(k p) c -> p k c",
                                                      p=128))

                def w8_load(t, src, c0, c1):
                    nc.scalar.dma_start(
                        out=t[:, :, c0:c1],
                        in_=src[:, c0:c1].rearrange("(k p) n -> p k n",
                                                    p=128))

                CH = min(512, T)
                xt8_load(0, CH)
                w8_load(wk8_t, wk8_d, 0, 512)
                w8_load(wk8_t, wk8_d, 512, 1024)
                for h in range(1, T // CH):
                    xt8_load(CH * h, CH * (h + 1))
                w8_load(wq8_t, wq8_d, 0, 1024)
                for h in range(T // CH):
                    csl = slice(CH * h, CH * (h + 1))
                    nc.sync.dma_start(
                        out=xt_all[:, :, csl],
                        in_=xt_d[:, csl].rearrange("(k p) c -> p k c", p=128))
                load_w(wv_t, "wv")
                load_w(wg_t, "wg")

                def dr_proj(pk, w8_t, t):
                    # contraction 1024 as 4 DoubleRow pair-slices of 256
                    for n in range(2):
                        nsl = slice(512 * n, 512 * (n + 1))
                        for s in range(KP):
                            nc.tensor.matmul(
                                pk[:, nsl],
                                lhsT=xt8_all[:, 2 * s:2 * s + 2,
                                             128 * t:128 * (t + 1)],
                                rhs=w8_t[:, 2 * s:2 * s + 2, nsl],
                                start=(s == 0), stop=(s == KP - 1),
                                perf_mode=DR,
                            )

                # ---- phase 1a: k projection + k_sum (k kept in SBUF) ------
                with (
                    tc.tile_pool(name="ks", bufs=1, space="PSUM") as kspool,
                    tc.tile_pool(name="pk", bufs=3, space="PSUM") as pkpool,
                ):
                    ks_ps = kspool.tile([1, H], F32)
                    assert NT % 2 == 0

                    def emit_ksum(j):
                        # DoubleRow pair: sums k tiles 2j and 2j+1 at once
                        for n in range(2):
                            nc.tensor.matmul(
                                ks_ps[:, 512 * n:512 * (n + 1)],
                                lhsT=ones8[:, :, 0:1],
                                rhs=kt_all[:, 2 * j:2 * j + 2,
                                           512 * n:512 * (n + 1)],
                                start=(j == 0),
                                stop=(j == NT // 2 - 1),
                                perf_mode=DR,
                            )

                    for t in range(NT):
                        pk = pkpool.tile([128, H], F32, tag="pk")
                        dr_proj(pk, wk8_t, t)
                        elu1(kt_all[:, t, :], pk)
                        # ksum of the previous tile pair: its elu chains
                        # finished while later tiles' matmuls ran
                        if t >= 2 and t % 2 == 0:
                            emit_ksum(t // 2 - 1)
                    emit_ksum(NT // 2 - 1)
                    with tc.high_priority():
                        ks_sb = singles.tile([1, H], F32)
                        nc.vector.tensor_copy(out=ks_sb, in_=ks_ps)
                # real AR chain, all on the gpsimd queue (the Sync queue
                # still drains phase-2 bulk loads); ksb stays f32 -- one
                # broadcast DMA, no convert hop (costs +0.5us/tile on the
                # phase-2 nprod mul, which has headroom).
                with tc.high_priority():
                    nc.gpsimd.dma_start(out=ks_in[:, :], in_=ks_sb)
                    nc.gpsimd.collective_compute(
                        "AllReduce", ALU.add,
                        replica_groups=[list(g) for g in groups],
                        ins=[ks_in[:, :]], outs=[ks_out[:, :]],
                    )
                    ksb = singles.tile([128, H], F32)
                    nc.gpsimd.dma_start(
                        out=ksb, in_=ks_out[0:1, :].to_broadcast([128, H]))
                if apply_beta:
                    beta_b = singles.tile([128, H], BF16)
                    nc.gpsimd.dma_start(
                        out=beta_b, in_=beta_d[0:1, :].to_broadcast([128, H]))

                # ---- phase 1b: q projection (q kept in SBUF; qk deferred) --
                # bufs=3: pq release needs exp+relu (scalar) AND the relu
                # half (DVE) -- a ~2.4us latency chain after the tile's MMs;
                # 2 bufs would bound the period at ~2.25us vs 2.1us of MMs.
                with tc.tile_pool(name="pq", bufs=3, space="PSUM") as pqpool:
                    for t in range(NT):
                        pq = pqpool.tile([128, H], F32, tag="pq")
                        dr_proj(pq, wq8_t, t)
                        elu1(qt_all[:, t, :], pq)

                # zero gate derived from 1b's last tile: phase-2 scalar ops
                # take it as bias so the scheduler cannot interleave them
                # into phase 1 (exp<->silu table thrash).
                gate0 = singles.tile([128, 1], F32)
                nc.vector.tensor_scalar(
                    out=gate0, in0=qt_all[:, NT - 1, 0:1],
                    scalar1=0.0, scalar2=None, op0=ALU.mult,
                )

            # ---------------- phase 2: v, g, z, LN, gate, Wo ----------------
            with ExitStack() as es2:
                pool2 = lambda n, b, **kw: es2.enter_context(
                    tc.tile_pool(name=n, bufs=b, **kw))
                wopool = pool2("wo", 1)
                vpool = pool2("vr", VBUFS)
                prodpool = pool2("prod", 1)
                zpool = pool2("z2", 3)
                spool = pool2("s2", PIPE + 1)
                upool = pool2("u2", PIPE + 1)
                utpool = pool2("ut", 3)
                ypool = pool2("y", 2)
                # rsig is consumed by back_end PIPE tiles later
                rspool = pool2("rs", PIPE + 1)
                # Wo + the v ring land in the bytes freed by xt8/wq8/wk8;
                # issued from the gpsimd queue (idle after the AR) because
                # the space frees only when 1b's last matmul retires -- a
                # blocked load on the scalar queue would deadlock behind the
                # gated silu.
                wo_t = wopool.tile([128, KT, H], BF16)
                nc.gpsimd.dma_start(
                    out=wo_t[:, :, :],
                    in_=w_d["wo"][:, :].rearrange("(k p) n -> p k n", p=128))
                # pool creation order controls PSUM bank placement: pa/pb
                # (needed at the first phase-2 matmul) grab the 4 banks that
                # were free during 1b; py (first needed ~35us in, PIPE deep)
                # takes the banks recycled from 1b's pq pool.
                if True:
                    papool = pool2("pa", 3, space="PSUM")
                    pbpool = pool2("pb", 3, space="PSUM")
                    pypool = pool2("py", 2, space="PSUM")

                    def back_end(u, rsig, t):
                        # u^T via the DMA XBAR hardware transpose (2-byte
                        # dtypes only); y = py * 1/sigma evacuates on ACT
                        # (Identity is in every table set -- no reload).
                        ut = utpool.tile([128, KT, 128], BF16, tag="ut")
                        nc.sync.dma_start_transpose(ut, u)
                        for n in range(2):
                            nsl = slice(512 * n, 512 * (n + 1))
                            py = pypool.tile([128, 512], F32, tag="py")
                            for k in range(KT):
                                mm(py, ut[:, k, :],
                                   wo_t[:, k, nsl], k == 0, k == KT - 1)
                            y_sb = ypool.tile([128, 512], BF16, tag="y")
                            if rsig is not None:
                                nc.scalar.activation(out=y_sb, in_=py,
                                                     func=ACT_F.Identity,
                                                     scale=rsig)
                            else:
                                nc.scalar.activation(out=y_sb, in_=py,
                                                     func=ACT_F.Identity)
                            nc.sync.dma_start(
                                out=out_d[128 * t:128 * (t + 1), nsl],
                                in_=y_sb)

                    # PIPE-deep software pipeline: tile t's back_end (wo
                    # matmuls) is enqueued at tile t+PIPE, giving the AR +
                    # ksb-gated DVE chain ~34us of PE runway at phase-2 start
                    prevs = []
                    for t in range(NT):
                        s_t = spool.tile([128, H], BF16, tag="s")
                        v_sb = vpool.tile([128, H], BF16, tag="v")
                        for n in range(2):
                            pv = papool.tile([128, 512], F32, tag="pa")
                            pg = pbpool.tile([128, 512], F32, tag="pb")
                            nsl = slice(512 * n, 512 * (n + 1))
                            for k in range(KT):
                                lhs = xt_all[:, k, 128 * t:128 * (t + 1)]
                                mm(pv, lhs, wv_t[:, k, nsl], k == 0, k == KT - 1)
                                mm(pg, lhs, wg_t[:, k, nsl], k == 0, k == KT - 1)
                            ssl = s_t[:, nsl]
                            if use_silu:
                                nc.scalar.activation(out=ssl, in_=pg,
                                                     func=ACT_F.Silu,
                                                     bias=gate0[:, 0:1])
                            else:  # CoreSim has no Silu table
                                nc.scalar.activation(out=ssl, in_=pg,
                                                     func=ACT_F.Sigmoid,
                                                     bias=gate0[:, 0:1])
                                nc.vector.tensor_mul(ssl, ssl, pg)
                            # v PSUM -> SBUF ring on ACT: frees pa so the
                            # v/g matmuls never wait on the ksb-gated DVE
                            nc.scalar.activation(out=v_sb[:, nsl], in_=pv,
                                                 func=ACT_F.Identity,
                                                 bias=gate0[:, 0:1])
                        # qk = per-head dot(q, k) -- deferred from 1b.  Both
                        # prod muls go through stt with the zero gate as the
                        # scalar: without it the scheduler front-runs them
                        # into 1b's DVE queue (they only need qt/kt tiles)
                        # and overloads it past the PE rate.
                        prod = prodpool.tile([128, H], BF16, tag="prod")
                        nc.vector.scalar_tensor_tensor(
                            out=prod, in0=qt_all[:, t, :], scalar=gate0[:, 0:1],
                            in1=kt_all[:, t, :], op0=ALU.add, op1=ALU.mult,
                        )
                        qk_t = smpool.tile([128, NH], F32, tag="qk")
                        nc.vector.reduce_sum(
                            out=qk_t,
                            in_=prod.rearrange("p (h d) -> p h d", d=DK),
                            axis=AX.X,
                        )
                        # normalizer = per-head dot(q, k_sum)
                        nprod = prodpool.tile([128, H], BF16, tag="prod")
                        nc.vector.scalar_tensor_tensor(
                            out=nprod, in0=qt_all[:, t, :], scalar=gate0[:, 0:1],
                            in1=ksb, op0=ALU.add, op1=ALU.mult,
                        )
                        norm = smpool.tile([128, NH], F32, tag="norm")
                        nc.vector.reduce_sum(
                            out=norm,
                            in_=nprod.rearrange("p (h d) -> p h d", d=DK),
                            axis=AX.X,
                        )
                        rec = smpool.tile([128, NH], F32, tag="rec")
                        nc.vector.tensor_scalar_add(out=rec, in0=norm,
                                                    scalar1=1e-6)
                        nc.vector.reciprocal(out=rec, in_=rec)
                        r = smpool.tile([128, NH], F32, tag="r")
                        nc.vector.tensor_mul(r, qk_t, rec)
                        # z = r (broadcast over d) * v
                        z = zpool.tile([128, H], BF16, tag="z")
                        for n in range(2):
                            rs = r[:, 8 * n:8 * (n + 1)]
                            r_b = bass.AP(tensor=rs.tensor, offset=rs.offset,
                                          ap=[list(rs.ap[0]), list(rs.ap[1]),
                                              [0, DK]])
                            nc.vector.tensor_tensor(
                                out=z[:, 512 * n:512 * (n + 1)],
                                in0=v_sb[:, 512 * n:512 * (n + 1)],
                                in1=r_b, op=ALU.mult,
                            )
                        # LayerNorm stats over the full 1024 features
                        st = smpool.tile([128, 2, nc.vector.BN_STATS_DIM], F32,
                                         tag="bnst")
                        for n in range(2):
                            nc.vector.bn_stats(out=st[:, n, :],
                                               in_=z[:, 512 * n:512 * (n + 1)])
                        mv = smpool.tile([128, nc.vector.BN_AGGR_DIM], F32,
                                         tag="mv")
                        nc.vector.bn_aggr(out=mv, in_=st)
                        # rsig = rsqrt(var + eps) on the DVE: exponent
                        # bit-hack seed + 1 Newton step (rel err ~1.7e-3,
                        # ~2e-4 on the output).  Off the critical path;
                        # consumed only at Wo PSUM evacuation.
                        vq = smpool.tile([128, 1], F32, tag="vq")
                        nc.vector.tensor_scalar_add(out=vq, in0=mv[:, 1:2],
                                                    scalar1=1e-5)
                        rsig = rspool.tile([128, 1], F32, tag="rsig")
                        nc.vector.tensor_tensor(
                            out=rsig.bitcast(U32), in0=vq.bitcast(U32),
                            in1=c_shift1, op=ALU.logical_shift_right,
                        )
                        nc.vector.tensor_tensor(
                            out=rsig.bitcast(U32), in0=c_magic,
                            in1=rsig.bitcast(U32), op=ALU.subtract,
                        )
                        nt1 = smpool.tile([128, 1], F32, tag="nt1")
                        nc.vector.tensor_mul(nt1, rsig, rsig)
                        nc.vector.tensor_mul(nt1, nt1, vq)
                        nc.vector.tensor_scalar(
                            out=nt1, in0=nt1, scalar1=-0.5, scalar2=1.5,
                            op0=ALU.mult, op1=ALU.add,
                        )
                        nc.vector.tensor_mul(rsig, rsig, nt1)
                        # u = (z - mu) * silu(g) in one stt; 1/sigma deferred
                        u = upool.tile([128, H], BF16, tag="u")
                        if apply_beta:
                            # beta breaks the deferral: apply rsig here
                            nc.vector.tensor_scalar(
                                out=u, in0=z, scalar1=mv[:, 0:1], scalar2=rsig,
                                op0=ALU.subtract, op1=ALU.mult,
                            )
                            nc.vector.tensor_add(out=u, in0=u, in1=beta_b)
                            nc.vector.tensor_mul(u, u, s_t)
                            rsig_eff = None
                        else:
                            nc.vector.scalar_tensor_tensor(
                                out=u, in0=z, scalar=mv[:, 0:1], in1=s_t,
                                op0=ALU.subtract, op1=ALU.mult,
                            )
                            rsig_eff = rsig
                        prevs.append((u, rsig_eff, t))
                        # full depth only while the AR needs runway; ramp
                        # down near the end so the tail doesn't bunch
                        depth = PIPE if t < NT - 2 else 2
                        while len(prevs) > depth:
                            back_end(*prevs.pop(0))
                    for p in prevs:
                        back_end(*p)
    return _split_multi_waits(nc) if split_waits else nc


# ------------------------------------------------------------------
# host glue
# ------------------------------------------------------------------
_CACHE = {}
LAST_RESULT = None


def kernel(hidden_states, Wq, Wk, Wv, Wg, Wo, gamma, beta):
    import ml_dtypes
    bf16 = ml_dtypes.bfloat16
    e4m3 = ml_dtypes.float8_e4m3

    hs = np.asarray(hidden_states, dtype=np.float32)
    Wq = np.asarray(Wq, dtype=np.float32)
    Wk = np.asarray(Wk, dtype=np.float32)
    Wv = np.asarray(Wv, dtype=np.float32)
    Wg = np.asarray(Wg, dtype=np.float32)
    Wo = np.asarray(Wo, dtype=np.float32)
    gamma = np.asarray(gamma, dtype=np.float32)
    beta = np.asarray(beta, dtype=np.float32)

    b, s, h = hs.shape
    tokens = hs.reshape(b * s, h)
    n_tok = b * s
    T = n_tok // N_CORES
    assert s % T == 0, "core token shards must not straddle batches"
    cores_per_batch = s // T

    groups = tuple(
        tuple(range(bi * cores_per_batch, (bi + 1) * cores_per_batch))
        for bi in range(b)
    )
    apply_beta = bool(np.any(beta))

    key = (T, groups, apply_beta)
    if key not in _CACHE:
        _CACHE[key] = build_gla(T=T, groups=groups, apply_beta=apply_beta)
    nc = _CACHE[key]

    wo_eff = (gamma[:, None] * Wo).astype(bf16)
    wq8 = (Wq * WSCALE).astype(e4m3)
    wk8 = (Wk * WSCALE).astype(e4m3)
    wv_b = Wv.astype(bf16)
    wg_b = Wg.astype(bf16)
    in_maps = []
    for i in range(N_CORES):
        xt_f32 = np.ascontiguousarray(tokens[i * T:(i + 1) * T].T)
        m = {
            "xt": xt_f32.astype(bf16),
            "xt8": xt_f32.astype(e4m3),
            "wq8": wq8, "wk8": wk8,
            "wv": wv_b, "wg": wg_b, "wo": wo_eff,
        }
        if apply_beta:
            m["beta"] = beta.reshape(1, h).astype(bf16)
        in_maps.append(m)

    res = run_bass_kernel_spmd(
        nc, in_maps, core_ids=list(range(N_CORES)),
        trace=bool(os.environ.get("GLA_TRACE")),
    )
    global LAST_RESULT
    LAST_RESULT = res
    out = np.concatenate(
        [res.results[i]["out"].astype(np.float32) for i in range(N_CORES)],
        axis=0)
    return out.reshape(b, s, h)


# revision 31
# speedup vs baseline: 1.0639x; 1.0402x over previous
"""Gated linear attention kernel for one TRN2 chip (8 NeuronCores).

Math (see reference):
    q = elu(X Wq)+1, k = elu(X Wk)+1, v = X Wv, g = X Wg
    qk = sum_d(q*k) per head; k_sum = sum_seq(k); norm = sum_d(q*k_sum)
    z = qk*v/(norm+1e-6); z = LayerNorm(z)*gamma+beta; out = (z*silu(g)) Wo

Sharding: data-parallel over the 16384 tokens, 2048 per core; cores 0-3 own
batch 0, cores 4-7 batch 1.  The only cross-core coupling is k_sum (a [1,1024]
vector per batch) -> AllReduce within 4-core groups.

The kernel is PE-streaming-bound at the GPIO-throttled 1.95 GHz clock (the
13/16 clock-gate engages ~60us in; MMs pipeline at 263ns/512cols), so v2/v3
cut PE cycles and then keep every other engine strictly under the PE:
  * q/k projections in fp8-e4m3 perf_mode=DoubleRow: 2 fp8 MACs/cell/cycle,
    pairing adjacent 128-row k-slices via 3D APs [128,2,M]/[128,2,N]
    (HW-verified, 8e-4).  Host pre-scales Wq/Wk by 32; the 1/32 descale folds
    into the elu ACTs' scale operand.  End-to-end rel err 6.9e-3 (gate 2e-2);
    the q-quantization error cancels between qk and norm, and k_sum averages
    8192 positive terms.  v/g/o cannot go fp8 (3.5e-2+ each, and u underflows
    e4m3 by ~2^-13).
  * elu = min(exp(x),1)+relu(x) on [128,1024] 2-bank PSUM tiles; exp + half
    the relu on ACT (1.87us/tile), other relu half + combine on DVE
    (1.87us/tile), both under the 2.1us/tile of DR matmuls.
  * qk = sum_d(q*k) deferred to phase 2 where the DVE has slack.
  * The AllReduce takes ~45us trigger-to-done on this stack (mesh latency +
    skew + a one-time ~50us replica-group barrier).  A dummy warm-up
    AllReduce issued at kernel start absorbs the barrier; the real AR chain
    (vector hp copy -> gpsimd dma -> AR -> vector hp f32->bf16 -> gpsimd
    broadcast) fires right after 1a.  Phase 2 gives it ~34us of runway: the
    software pipeline is 4 deep and v PSUM evacuates to a 6-tile rolling
    SBUF buffer via scalar Identity, so v/g matmuls never wait on the
    ksb-gated DVE chain.
  * Phase-2 scalar ops (silu + v-evac) are hard-gated on a zero-bias AP
    derived from 1b's last tile: without it the scheduler interleaves silu
    into phase 1 and thrashes the exp<->silu ACT tables (8x1.3us reloads on
    the bottleneck engine).  Identity/relu live in every table set.
  * y = py*rsig evacuates on ACT (Identity, scale=rsig); u = (z-mu)*s in one
    stt; rsqrt uses 1 Newton step (rel err 1.7e-3 on rsig, ~0.2e-3 on out).
  * SBUF lifetimes: xt8/wq8/wk8 close after 1b; Wo + the v ring live in the
    freed bytes (Wo loads via the gpsimd queue -- a blocked load on the
    scalar queue would deadlock behind the gated silu).
  * Output is stored bf16 (halves write traffic), upcast on the host.
Carried over from v1: X^T/k/q SBUF-resident, DVE rsqrt Newton with 1/sigma
folded past Wo, DMA-XBAR transpose for u^T, k-sliced initial loads, gamma
folded into Wo on the host; beta==0 verified on the host.
"""

import os
from contextlib import ExitStack

import numpy as np

import concourse.bass as bass
import concourse.mybir as mybir
import concourse.tile as tile
from concourse.bass_utils import run_bass_kernel_spmd

F32 = mybir.dt.float32
BF16 = mybir.dt.bfloat16
FP8 = mybir.dt.float8e4
U32 = mybir.dt.uint32
AX = mybir.AxisListType
ALU = mybir.AluOpType
ACT_F = mybir.ActivationFunctionType
DR = mybir.MatmulPerfMode.DoubleRow

H = 1024
NH = 16
DK = 64
N_CORES = 8
WSCALE = 32.0          # host multiplies Wq/Wk by this before e4m3 quantization
ISCALE = 1.0 / WSCALE  # folded into the elu ACTs
PIPE = 4               # phase-2 software pipeline depth (AR runway)
VBUFS = 6              # rolling v ring tiles


def _split_multi_waits(nc, cap=1):
    """walrus in this image rejects instructions with more than ~2 sync waits
    (Tile attaches several to its kernel-tail drain).  Move excess waits onto
    preceding same-engine NoOps."""
    for f in nc.m.functions:
        for bb in f.blocks:
            insts = bb.instructions
            new_list = []
            changed = False
            for inst in insts:
                si = inst.sync_info
                waits = list(si.on_wait) if si else []
                if len(waits) > cap:
                    changed = True
                    for kk, w in enumerate(waits[:-cap]):
                        new_list.append(
                            mybir.InstNoOp(
                                name=f"{inst.name}-wsplit{kk}",
                                engine=inst.engine,
                                ins=[],
                                outs=[],
                                sync_info=mybir.SyncInfo(on_wait=[w], on_update=[]),
                            )
                        )
                    inst.sync_info = mybir.SyncInfo(
                        on_wait=waits[-cap:], on_update=list(si.on_update)
                    )
                new_list.append(inst)
            if changed:
                live = bb.instructions
                live.clear()
                for i in new_list:
                    bb.add_instruction(i)
    return nc


def build_gla(T=2048, groups=((0, 1, 2, 3), (4, 5, 6, 7)), n_devices=8,
              apply_beta=False, split_waits=True, use_silu=True):
    """Build the per-core SPMD program.  T = tokens per core."""
    assert T % 128 == 0
    NT = T // 128      # 128-token tiles
    KT = H // 128      # contraction slices
    KP = KT // 2       # DoubleRow k-pair slices

    nc = bass.Bass(num_devices=n_devices)
    xt_d = nc.declare_dram_parameter("xt", [H, T], BF16, isOutput=False)
    xt8_d = nc.declare_dram_parameter("xt8", [H, T], FP8, isOutput=False)
    wq8_d = nc.declare_dram_parameter("wq8", [H, H], FP8, isOutput=False)
    wk8_d = nc.declare_dram_parameter("wk8", [H, H], FP8, isOutput=False)
    w_d = {
        n: nc.declare_dram_parameter(n, [H, H], BF16, isOutput=False)
        for n in ("wv", "wg", "wo")
    }
    beta_d = (
        nc.declare_dram_parameter("beta", [1, H], BF16, isOutput=False)
        if apply_beta
        else None
    )
    out_d = nc.declare_dram_parameter("out", [T, H], BF16, isOutput=True)

    ks_in = nc.dram_tensor("ks_in", [1, H], F32)
    ks_out = nc.dram_tensor("ks_out", [1, H], F32)
    arw_in = nc.dram_tensor("arw_in", [1, 8], F32)
    arw_out = nc.dram_tensor("arw_out", [1, 8], F32)

    def mm(ps, lhsT, rhs, start, stop):
        nc.tensor.matmul(ps, lhsT=lhsT, rhs=rhs, start=start, stop=stop)

    with tile.TileContext(nc) as tc:
        with (
            tc.tile_pool(name="singles", bufs=1) as singles,
            tc.tile_pool(name="w", bufs=2) as wpool,
            tc.tile_pool(name="xt", bufs=1) as xtpool,
            tc.tile_pool(name="kt", bufs=1) as ktpool,
            tc.tile_pool(name="qt", bufs=1) as qtpool,
            tc.tile_pool(name="small", bufs=3) as smpool,
        ):
            # fp8 ones pair for the DoubleRow k_sum (the 16-wide tile keeps
            # the pair-axis byte step at 16, a DR weight-AP constraint)
            ones8 = singles.tile([128, 2, 16], FP8)
            nc.vector.memset(ones8, 1.0)
            # rsqrt bit-hack constants (as APs: immediate ints on uint ops
            # are unreliable through the f32 immediate path)
            c_shift1 = singles.tile([128, 1], U32)
            nc.vector.memset(c_shift1, 1)
            c_magic = singles.tile([128, 1], U32)
            nc.vector.memset(c_magic, 0x5F3759DF)

            # warm-up AllReduce: establishes the replica-group barrier +
            # CC stream (~50us, one-time) while phase 1 computes, so the
            # real k_sum AR only pays ring latency.
            arw_sb = singles.tile([1, 8], F32)
            nc.gpsimd.memset(arw_sb, 0.0)
            nc.gpsimd.dma_start(out=arw_in[:, :], in_=arw_sb)
            nc.gpsimd.collective_compute(
                "AllReduce", ALU.add,
                replica_groups=[list(g) for g in groups],
                ins=[arw_in[:, :]], outs=[arw_out[:, :]],
            )

            # k is stored fp8: the elu stt writes e4m3 directly, k_sum runs
            # as DoubleRow tile-pairs (half the PE cycles), and the same
            # quantized k feeds qk AND (via k_sum) norm, so the error mostly
            # cancels in the ratio -- host-simulated rel err 8.2e-3.
            xt_all = xtpool.tile([128, KT, T], BF16)
            kt_all = ktpool.tile([128, NT, H], FP8)
            qt_all = qtpool.tile([128, NT, H], BF16)
            wv_t = wpool.tile([128, KT, H], BF16, tag="w", name="wv")
            wg_t = wpool.tile([128, KT, H], BF16, tag="w", name="wg")

            def load_w(t, name, engine=None):
                # one wide descriptor: [H, H] viewed as [p, k-slice, cols]
                (engine or nc.sync).dma_start(
                    out=t[:, :, :],
                    in_=w_d[name][:, :].rearrange("(k p) n -> p k n", p=128))

            def elu1(dst, ps):
                # dst = elu(ps/32)+1 = min(exp(ps/32), 1) + relu(ps/32);
                # exp + low relu half on ACT, high relu half + combine on DVE
                e = elupool.tile([128, H], BF16, tag="elue")
                r = elupool.tile([128, H], BF16, tag="elur")
                nc.scalar.activation(out=e, in_=ps, func=ACT_F.Exp,
                                     scale=ISCALE)
                nc.scalar.activation(out=r[:, 0:512], in_=ps[:, 0:512],
                                     func=ACT_F.Relu, scale=ISCALE)
                nc.vector.tensor_scalar(
                    out=r[:, 512:H], in0=ps[:, 512:H],
                    scalar1=ISCALE, scalar2=0.0, op0=ALU.mult, op1=ALU.max,
                )
                nc.vector.scalar_tensor_tensor(
                    out=dst, in0=e, scalar=1.0, in1=r,
                    op0=ALU.min, op1=ALU.add,
                )

            with (
                tc.tile_pool(name="x8", bufs=1) as xt8pool,
                tc.tile_pool(name="w8", bufs=2) as w8pool,
                tc.tile_pool(name="elu", bufs=3) as elupool,
            ):
                xt8_all = xt8pool.tile([128, KT, T], FP8)
                wk8_t = w8pool.tile([128, KT, H], FP8, tag="w8", name="wk8")
                wq8_t = w8pool.tile([128, KT, H], FP8, tag="w8", name="wq8")

                # staged initial loads, wide rearranged descriptors: tile 0
                # can start after ~1 MB (xt8+wk8 column halves); phase-2
                # inputs follow on the Sync queue.
                def xt8_load(c0, c1):
                    nc.sync.dma_start(
                        out=xt8_all[:, :, c0:c1],
                        in_=xt8_d[:, c0:c1].rearrange("(k p) c -> p k c",
                                                      p=128))

                def w8_load(t, src, c0, c1):
                    nc.scalar.dma_start(
                        out=t[:, :, c0:c1],
                        in_=src[:, c0:c1].rearrange("(k p) n -> p k n",
                                                    p=128))

                CH = min(512, T)
                xt8_load(0, CH)
                w8_load(wk8_t, wk8_d, 0, 512)
                w8_load(wk8_t, wk8_d, 512, 1024)
                for h in range(1, T // CH):
                    xt8_load(CH * h, CH * (h + 1))
                w8_load(wq8_t, wq8_d, 0, 1024)
                for h in range(T // CH):
                    csl = slice(CH * h, CH * (h + 1))
                    nc.sync.dma_start(
                        out=xt_all[:, :, csl],
                        in_=xt_d[:, csl].rearrange("(k p) c -> p k c", p=128))
                load_w(wv_t, "wv")
                load_w(wg_t, "wg")

                def dr_proj(pk, w8_t, t):
                    # contraction 1024 as 4 DoubleRow pair-slices of 256
                    for n in range(2):
                        nsl = slice(512 * n, 512 * (n + 1))
                        for s in range(KP):
                            nc.tensor.matmul(
                                pk[:, nsl],
                                lhsT=xt8_all[:, 2 * s:2 * s + 2,
                                             128 * t:128 * (t + 1)],
                                rhs=w8_t[:, 2 * s:2 * s + 2, nsl],
                                start=(s == 0), stop=(s == KP - 1),
                                perf_mode=DR,
                            )

                # ---- phase 1a: k projection + k_sum (k kept in SBUF) ------
                with (
                    tc.tile_pool(name="ks", bufs=1, space="PSUM") as kspool,
                    tc.tile_pool(name="pk", bufs=3, space="PSUM") as pkpool,
                ):
                    ks_ps = kspool.tile([1, H], F32)
                    assert NT % 2 == 0

                    def emit_ksum(j):
                        # DoubleRow pair: sums k tiles 2j and 2j+1 at once
                        for n in range(2):
                            nc.tensor.matmul(
                                ks_ps[:, 512 * n:512 * (n + 1)],
                                lhsT=ones8[:, :, 0:1],
                                rhs=kt_all[:, 2 * j:2 * j + 2,
                                           512 * n:512 * (n + 1)],
                                start=(j == 0),
                                stop=(j == NT // 2 - 1),
                                perf_mode=DR,
                            )

                    for t in range(NT):
                        pk = pkpool.tile([128, H], F32, tag="pk")
                        dr_proj(pk, wk8_t, t)
                        # the last tiles' elu chains gate the 1a->1b
                        # boundary AND the k_sum -> AllReduce trigger
                        if t >= NT - 2:
                            with tc.high_priority():
                                elu1(kt_all[:, t, :], pk)
                        else:
                            elu1(kt_all[:, t, :], pk)
                        # ksum lags two tile-pairs: pair j's stt must have
                        # fully drained or the PE waits ~0.7us on the DVE
                        if t >= 4 and t % 2 == 0:
                            emit_ksum(t // 2 - 2)
                    emit_ksum(NT // 2 - 2)
                    emit_ksum(NT // 2 - 1)
                    with tc.high_priority():
                        ks_sb = singles.tile([1, H], F32)
                        nc.vector.tensor_copy(out=ks_sb, in_=ks_ps)
                # real AR chain, all on the gpsimd queue (the Sync queue
                # still drains phase-2 bulk loads); ksb stays f32 -- one
                # broadcast DMA, no convert hop (costs +0.5us/tile on the
                # phase-2 nprod mul, which has headroom).
                with tc.high_priority():
                    nc.gpsimd.dma_start(out=ks_in[:, :], in_=ks_sb)
                    nc.gpsimd.collective_compute(
                        "AllReduce", ALU.add,
                        replica_groups=[list(g) for g in groups],
                        ins=[ks_in[:, :]], outs=[ks_out[:, :]],
                    )
                    ksb = singles.tile([128, H], F32)
                    nc.gpsimd.dma_start(
                        out=ksb, in_=ks_out[0:1, :].to_broadcast([128, H]))
                if apply_beta:
                    beta_b = singles.tile([128, H], BF16)
                    nc.gpsimd.dma_start(
                        out=beta_b, in_=beta_d[0:1, :].to_broadcast([128, H]))

                # ---- phase 1b: q projection (q kept in SBUF; qk deferred) --
                # bufs=3: pq release needs exp+relu (scalar) AND the relu
                # half (DVE) -- a ~2.4us latency chain after the tile's MMs;
                # 2 bufs would bound the period at ~2.25us vs 2.1us of MMs.
                with tc.tile_pool(name="pq", bufs=3, space="PSUM") as pqpool:
                    for t in range(NT):
                        pq = pqpool.tile([128, H], F32, tag="pq")
                        dr_proj(pq, wq8_t, t)
                        if t >= NT - 2:  # gates the 1b->2 boundary
                            with tc.high_priority():
                                elu1(qt_all[:, t, :], pq)
                        else:
                            elu1(qt_all[:, t, :], pq)

                # zero gate derived from 1b's last tile: phase-2 scalar ops
                # take it as bias so the scheduler cannot interleave them
                # into phase 1 (exp<->silu table thrash).
                gate0 = singles.tile([128, 1], F32)
                nc.vector.tensor_scalar(
                    out=gate0, in0=qt_all[:, NT - 1, 0:1],
                    scalar1=0.0, scalar2=None, op0=ALU.mult,
                )

            # ---------------- phase 2: v, g, z, LN, gate, Wo ----------------
            with ExitStack() as es2:
                pool2 = lambda n, b, **kw: es2.enter_context(
                    tc.tile_pool(name=n, bufs=b, **kw))
                wopool = pool2("wo", 1)
                vpool = pool2("vr", VBUFS)
                prodpool = pool2("prod", 1)
                zpool = pool2("z2", 3)
                spool = pool2("s2", PIPE + 1)
                upool = pool2("u2", PIPE + 1)
                utpool = pool2("ut", 3)
                ypool = pool2("y", 2)
                # rsig is consumed by back_end PIPE tiles later
                rspool = pool2("rs", PIPE + 1)
                # Wo + the v ring land in the bytes freed by xt8/wq8/wk8;
                # issued from the gpsimd queue (idle after the AR) because
                # the space frees only when 1b's last matmul retires -- a
                # blocked load on the scalar queue would deadlock behind the
                # gated silu.
                wo_t = wopool.tile([128, KT, H], BF16)
                nc.gpsimd.dma_start(
                    out=wo_t[:, :, :],
                    in_=w_d["wo"][:, :].rearrange("(k p) n -> p k n", p=128))
                # pool creation order controls PSUM bank placement: pa/pb
                # (needed at the first phase-2 matmul) grab the 4 banks that
                # were free during 1b; py (first needed ~35us in, PIPE deep)
                # takes the banks recycled from 1b's pq pool.
                if True:
                    papool = pool2("pa", 3, space="PSUM")
                    pbpool = pool2("pb", 3, space="PSUM")
                    pypool = pool2("py", 2, space="PSUM")

                    def back_end(u, rsig, t):
                        # u^T via the DMA XBAR hardware transpose (2-byte
                        # dtypes only); y = py * 1/sigma evacuates on ACT
                        # (Identity is in every table set -- no reload).
                        ut = utpool.tile([128, KT, 128], BF16, tag="ut")
                        nc.sync.dma_start_transpose(ut, u)
                        for n in range(2):
                            nsl = slice(512 * n, 512 * (n + 1))
                            py = pypool.tile([128, 512], F32, tag="py")
                            for k in range(KT):
                                mm(py, ut[:, k, :],
                                   wo_t[:, k, nsl], k == 0, k == KT - 1)
                            y_sb = ypool.tile([128, 512], BF16, tag="y")
                            if rsig is not None:
                                nc.scalar.activation(out=y_sb, in_=py,
                                                     func=ACT_F.Identity,
                                                     scale=rsig)
                            else:
                                nc.scalar.activation(out=y_sb, in_=py,
                                                     func=ACT_F.Identity)
                            nc.sync.dma_start(
                                out=out_d[128 * t:128 * (t + 1), nsl],
                                in_=y_sb)

                    # PIPE-deep software pipeline: tile t's back_end (wo
                    # matmuls) is enqueued at tile t+PIPE, giving the AR +
                    # ksb-gated DVE chain ~34us of PE runway at phase-2 start
                    prevs = []
                    for t in range(NT):
                        s_t = spool.tile([128, H], BF16, tag="s")
                        v_sb = vpool.tile([128, H], BF16, tag="v")
                        for n in range(2):
                            pv = papool.tile([128, 512], F32, tag="pa")
                            pg = pbpool.tile([128, 512], F32, tag="pb")
                            nsl = slice(512 * n, 512 * (n + 1))
                            for k in range(KT):
                                lhs = xt_all[:, k, 128 * t:128 * (t + 1)]
                                mm(pv, lhs, wv_t[:, k, nsl], k == 0, k == KT - 1)
                                mm(pg, lhs, wg_t[:, k, nsl], k == 0, k == KT - 1)
                            ssl = s_t[:, nsl]
                            if use_silu:
                                nc.scalar.activation(out=ssl, in_=pg,
                                                     func=ACT_F.Silu,
                                                     bias=gate0[:, 0:1])
                            else:  # CoreSim has no Silu table
                                nc.scalar.activation(out=ssl, in_=pg,
                                                     func=ACT_F.Sigmoid,
                                                     bias=gate0[:, 0:1])
                                nc.vector.tensor_mul(ssl, ssl, pg)
                            # v PSUM -> SBUF ring on ACT: frees pa so the
                            # v/g matmuls never wait on the ksb-gated DVE
                            nc.scalar.activation(out=v_sb[:, nsl], in_=pv,
                                                 func=ACT_F.Identity,
                                                 bias=gate0[:, 0:1])
                        # qk = per-head dot(q, k) -- deferred from 1b.  Both
                        # prod muls go through stt with the zero gate as the
                        # scalar: without it the scheduler front-runs them
                        # into 1b's DVE queue (they only need qt/kt tiles)
                        # and overloads it past the PE rate.
                        prod = prodpool.tile([128, H], BF16, tag="prod")
                        nc.vector.scalar_tensor_tensor(
                            out=prod, in0=qt_all[:, t, :], scalar=gate0[:, 0:1],
                            in1=kt_all[:, t, :], op0=ALU.add, op1=ALU.mult,
                        )
                        qk_t = smpool.tile([128, NH], F32, tag="qk")
                        nc.vector.reduce_sum(
                            out=qk_t,
                            in_=prod.rearrange("p (h d) -> p h d", d=DK),
                            axis=AX.X,
                        )
                        # normalizer = per-head dot(q, k_sum)
                        nprod = prodpool.tile([128, H], BF16, tag="prod")
                        nc.vector.scalar_tensor_tensor(
                            out=nprod, in0=qt_all[:, t, :], scalar=gate0[:, 0:1],
                            in1=ksb, op0=ALU.add, op1=ALU.mult,
                        )
                        norm = smpool.tile([128, NH], F32, tag="norm")
                        nc.vector.reduce_sum(
                            out=norm,
                            in_=nprod.rearrange("p (h d) -> p h d", d=DK),
                            axis=AX.X,
                        )
                        rec = smpool.tile([128, NH], F32, tag="rec")
                        nc.vector.tensor_scalar_add(out=rec, in0=norm,
                                                    scalar1=1e-6)
                        nc.vector.reciprocal(out=rec, in_=rec)
                        r = smpool.tile([128, NH], F32, tag="r")
                        nc.vector.tensor_mul(r, qk_t, rec)
                        # z = r (broadcast over d) * v
                        z = zpool.tile([128, H], BF16, tag="z")
                        for n in range(2):
                            rs = r[:, 8 * n:8 * (n + 1)]
                            r_b = bass.AP(tensor=rs.tensor, offset=rs.offset,
                                          ap=[list(rs.ap[0]), list(rs.ap[1]),
                                              [0, DK]])
                            nc.vector.tensor_tensor(
                                out=z[:, 512 * n:512 * (n + 1)],
                                in0=v_sb[:, 512 * n:512 * (n + 1)],
                                in1=r_b, op=ALU.mult,
                            )
                        # LayerNorm stats over the full 1024 features
                        st = smpool.tile([128, 2, nc.vector.BN_STATS_DIM], F32,
                                         tag="bnst")
                        for n in range(2):
                            nc.vector.bn_stats(out=st[:, n, :],
                                               in_=z[:, 512 * n:512 * (n + 1)])
                        mv = smpool.tile([128, nc.vector.BN_AGGR_DIM], F32,
                                         tag="mv")
                        nc.vector.bn_aggr(out=mv, in_=st)
                        # rsig = rsqrt(var + eps) on the DVE: exponent
                        # bit-hack seed + 1 Newton step (rel err ~1.7e-3,
                        # ~2e-4 on the output).  Off the critical path;
                        # consumed only at Wo PSUM evacuation.
                        vq = smpool.tile([128, 1], F32, tag="vq")
                        nc.vector.tensor_scalar_add(out=vq, in0=mv[:, 1:2],
                                                    scalar1=1e-5)
                        rsig = rspool.tile([128, 1], F32, tag="rsig")
                        nc.vector.tensor_tensor(
                            out=rsig.bitcast(U32), in0=vq.bitcast(U32),
                            in1=c_shift1, op=ALU.logical_shift_right,
                        )
                        nc.vector.tensor_tensor(
                            out=rsig.bitcast(U32), in0=c_magic,
                            in1=rsig.bitcast(U32), op=ALU.subtract,
                        )
                        nt1 = smpool.tile([128, 1], F32, tag="nt1")
                        nc.vector.tensor_mul(nt1, rsig, rsig)
                        nc.vector.tensor_mul(nt1, nt1, vq)
                        nc.vector.tensor_scalar(
                            out=nt1, in0=nt1, scalar1=-0.5, scalar2=1.5,
                            op0=ALU.mult, op1=ALU.add,
                        )
                        nc.vector.tensor_mul(rsig, rsig, nt1)
                        # u = (z - mu) * silu(g) in one stt; 1/sigma deferred
                        u = upool.tile([128, H], BF16, tag="u")
                        if apply_beta:
                            # beta breaks the deferral: apply rsig here
                            nc.vector.tensor_scalar(
                                out=u, in0=z, scalar1=mv[:, 0:1], scalar2=rsig,
                                op0=ALU.subtract, op1=ALU.mult,
                            )
                            nc.vector.tensor_add(out=u, in0=u, in1=beta_b)
                            nc.vector.tensor_mul(u, u, s_t)
                            rsig_eff = None
                        else:
                            nc.vector.scalar_tensor_tensor(
                                out=u, in0=z, scalar=mv[:, 0:1], in1=s_t,
                                op0=ALU.subtract, op1=ALU.mult,
                            )
                            rsig_eff = rsig
                        prevs.append((u, rsig_eff, t))
                        # full depth only while the AR needs runway; ramp
                        # down near the end so the tail doesn't bunch
                        depth = PIPE if t < NT - 2 else 2
                        while len(prevs) > depth:
                            back_end(*prevs.pop(0))
                    for p in prevs:
                        back_end(*p)
    return _split_multi_waits(nc) if split_waits else nc


# ------------------------------------------------------------------
# host glue
# ------------------------------------------------------------------
_CACHE = {}
LAST_RESULT = None


def kernel(hidden_states, Wq, Wk, Wv, Wg, Wo, gamma, beta):
    import ml_dtypes
    bf16 = ml_dtypes.bfloat16
    e4m3 = ml_dtypes.float8_e4m3

    hs = np.asarray(hidden_states, dtype=np.float32)
    Wq = np.asarray(Wq, dtype=np.float32)
    Wk = np.asarray(Wk, dtype=np.float32)
    Wv = np.asarray(Wv, dtype=np.float32)
    Wg = np.asarray(Wg, dtype=np.float32)
    Wo = np.asarray(Wo, dtype=np.float32)
    gamma = np.asarray(gamma, dtype=np.float32)
    beta = np.asarray(beta, dtype=np.float32)

    b, s, h = hs.shape
    tokens = hs.reshape(b * s, h)
    n_tok = b * s
    T = n_tok // N_CORES
    assert s % T == 0, "core token shards must not straddle batches"
    cores_per_batch = s // T

    groups = tuple(
        tuple(range(bi * cores_per_batch, (bi + 1) * cores_per_batch))
        for bi in range(b)
    )
    apply_beta = bool(np.any(beta))

    key = (T, groups, apply_beta)
    if key not in _CACHE:
        _CACHE[key] = build_gla(T=T, groups=groups, apply_beta=apply_beta)
    nc = _CACHE[key]

    wo_eff = (gamma[:, None] * Wo).astype(bf16)
    wq8 = (Wq * WSCALE).astype(e4m3)
    wk8 = (Wk * WSCALE).astype(e4m3)
    wv_b = Wv.astype(bf16)
    wg_b = Wg.astype(bf16)
    in_maps = []
    for i in range(N_CORES):
        xt_f32 = np.ascontiguousarray(tokens[i * T:(i + 1) * T].T)
        m = {
            "xt": xt_f32.astype(bf16),
            "xt8": xt_f32.astype(e4m3),
            "wq8": wq8, "wk8": wk8,
            "wv": wv_b, "wg": wg_b, "wo": wo_eff,
        }
        if apply_beta:
            m["beta"] = beta.reshape(1, h).astype(bf16)
        in_maps.append(m)

    res = run_bass_kernel_spmd(
        nc, in_maps, core_ids=list(range(N_CORES)),
        trace=bool(os.environ.get("GLA_TRACE")),
    )
    global LAST_RESULT
    LAST_RESULT = res
    out = np.concatenate(
        [res.results[i]["out"].astype(np.float32) for i in range(N_CORES)],
        axis=0)
    return out.reshape(b, s, h)
